# revision 4
# baseline (speedup 1.0000x reference)
"""Trainium (trn2) Bass kernel for a 2-layer GAT over N=100k nodes / E=1.7M edges.

Strategy (v2 — node-transform / edge-aggregate split)
-----------------------------------------------------
Edges are sorted by destination on the host (index-only preprocessing); the
destination axis is sharded across the 8 NeuronCores in contiguous 128-node
windows (98 windows per core).  Each GAT layer runs as TWO SPMD kernels with
host-side index gathers (pure permutations / casts — no host FLOPs) between
them:

* node kernel (P):  h = x @ W and the folded attention logits
  al_s = x @ (W a_s), al_d = x @ (W a_d) are computed ONCE PER NODE
  (dense matmuls, ~25 us/core).  For layer 2 the ELU of the layer-1
  output is fused into this kernel's input stream.
* host: gathers per-edge streams h[src], al_s[src], al_d[dst] into the
  dst-sorted slot order (numpy fancy indexing = permutation only).
* edge kernel (E):  per 128-edge tile, z = al_s+al_d (DVE), leaky_relu &
  exp on the Scalar engine (constant -4 bias keeps fp16 exp in range and
  cancels in the softmax), messages m = h_src * exp(z) (DVE, with the
  exp broadcast pre-expanded by a Scalar-engine copy so the multiply
  runs in 2x mode), and a single matmul per tile accumulates both the
  numerator segment-sum and the denominators into one PSUM slot via an
  on-chip selection matrix S[e,n] = (rel_dst[e]==n) built with one
  tensor_scalar(is_equal).  Layer 2 has 1 head, so exp(z) is folded
  directly into S by a dual-op tensor_scalar (is_equal, mult) and the
  message multiply disappears; the denominator rides on a host-appended
  ones column of the feature stream.

vs the v1 kernel this removes the per-edge recompute of x[src] @ W (17x the
node-phase FLOPs), the streamed one-hot S^T matrix (54 MB/core of HBM reads)
and 3 of the 4 per-tile matmuls; the edge kernels are Vector-engine bound at
~1 matmul + ~1.5 DVE ops per 128-edge tile.

Environment workarounds: this container's walrus build allows only ONE
semaphore wait per instruction (split onto nop carriers post-scheduling), and
the GPSIMD ucode libraries are absent (so no dma_gather/indirect-DMA fast
paths - hence the host-gather design).
"""
import numpy as np

import concourse.bass as bass
import concourse.mybir as mybir
import concourse.tile as tile
from concourse.bass_utils import run_bass_kernel_spmd

P = 128
F16 = mybir.dt.float16
F32 = mybir.dt.float32
AF = mybir.ActivationFunctionType
OP = mybir.AluOpType
NEG_SLOPE = 0.2
EXP_BIAS = -4.0     # exp(z + EXP_BIAS): constant shift cancels in softmax
GRP = 16            # tiles per stream group
PAD_REL = 255.0     # rel value for pad slots -> is_equal never matches
N_CORES = 8
EPS = 1e-30
CH = 512            # node-kernel chunk (one PSUM bank of fp32)

# ------------------------------------------------------------------ patches

_wsplit_counter = [0]


def _split_excess_waits(nc, max_waits=1):
    """This walrus build rejects >1 sem-wait per instruction ("Too many sync
    wait commands"). Move overflow waits onto same-engine nop carriers."""
    n_split = 0
    for f in nc.m.functions:
        for blk in f.blocks:
            changed = False
            out = []
            for inst in blk.instructions:
                si = inst.sync_info
                if si is not None and len(si.on_wait) > max_waits:
                    waits = list(si.on_wait)
                    keep = waits[len(waits) - max_waits:]
                    overflow = waits[: len(waits) - max_waits]
                    for i in range(0, len(overflow), max_waits):
                        _wsplit_counter[0] += 1
                        nop = mybir.InstNoOp(
                            name=f"I-wsplit-{_wsplit_counter[0]}", ins=[], outs=[])
                        nop.engine = inst.engine
                        nop.sync_info = mybir.SyncInfo(
                            on_wait=overflow[i: i + max_waits], on_update=[])
                        out.append(nop)
                    inst.sync_info = mybir.SyncInfo(
                        on_wait=keep, on_update=list(si.on_update))
                    changed = True
                    n_split += 1
                out.append(inst)
            if changed:
                blk.instructions = out
    return n_split


def _finalize_kernel(nc):
    import bass_rust as _bass_rust
    from concourse.library_config import all_libraries, standard
    from concourse.library_overlay import lower_extended_insts

    inst_type_to_lib_mask = {}
    for lib in all_libraries:
        for inst_type in lib.instructions:
            inst_type_to_lib_mask[inst_type] = inst_type_to_lib_mask.get(
                inst_type, 0) | (1 << lib.index)
    _bass_rust.insert_library_loads(
        nc, inst_type_to_lib_mask, len(all_libraries), standard.index)
    lower_extended_insts(nc)
    _split_excess_waits(nc)


# ------------------------------------------------------------------ host prep

class _Graph:
    """Host-side index preprocessing: sort by dst, shard dst windows across
    cores, pad per-window tile counts to a global schedule so all cores run
    one identical SPMD program."""

    def __init__(self, edge_index, n_nodes, n_cores):
        self.N = n_nodes
        self.C = n_cores
        src = np.asarray(edge_index[0], dtype=np.int64)
        dst = np.asarray(edge_index[1], dtype=np.int64)
        perm = np.argsort(dst, kind="stable")
        self.src_s = src[perm].astype(np.int32)
        self.dst_s = dst[perm].astype(np.int32)

        n_win_total = (n_nodes + P - 1) // P
        self.wpc = (n_win_total + n_cores - 1) // n_cores
        self.n_win = self.wpc * n_cores
        self.shard_nodes = self.wpc * P
        self.n_pad = self.n_win * P

        bounds = np.searchsorted(self.dst_s, np.arange(0, self.n_win + 1) * P)
        counts = np.zeros((n_cores, self.wpc), dtype=np.int64)
        for k in range(n_cores):
            for i in range(self.wpc):
                w = k * self.wpc + i
                if w < n_win_total:
                    counts[k, i] = bounds[w + 1] - bounds[w]
        self.PC = np.maximum(np.ceil(counts / P).astype(np.int64).max(axis=0), 1)
        self.T = int(self.PC.sum())

        self.slot_src = np.zeros((n_cores, self.T * P), dtype=np.int32)
        self.slot_dst = np.zeros((n_cores, self.T * P), dtype=np.int32)
        self.slot_rel = np.full((n_cores, self.T * P), int(PAD_REL), dtype=np.int32)
        for k in range(n_cores):
            t0 = 0
            for i in range(self.wpc):
                w = k * self.wpc + i
                cnt = int(counts[k, i])
                if cnt > 0:
                    e0 = bounds[w]
                    sl = t0 * P
                    self.slot_src[k, sl:sl + cnt] = self.src_s[e0:e0 + cnt]
                    self.slot_dst[k, sl:sl + cnt] = self.dst_s[e0:e0 + cnt]
                    self.slot_rel[k, sl:sl + cnt] = self.dst_s[e0:e0 + cnt] - w * P
                t0 += int(self.PC[i])
        self.src2d = self.slot_src.reshape(n_cores, self.T, P)
        self.dst2d = self.slot_dst.reshape(n_cores, self.T, P)
        self.rel2d = self.slot_rel.reshape(n_cores, self.T, P)

    def stream_feat(self, table, core, ones_col=False):
        """[128, T*C] (or T*(C+1) with a trailing ones column per tile):
        col t*C+c of partition e = table[src[slot t,e], c]."""
        T, C = self.T, table.shape[1]
        W = C + 1 if ones_col else C
        out = np.empty((T, P, W), dtype=np.float16)
        out[:, :, :C] = table[self.src2d[core]]
        if ones_col:
            out[:, :, C] = 1.0
        return np.ascontiguousarray(out.transpose(1, 0, 2)).reshape(P, T * W)

    def stream_zs(self, als, ald, core):
        """[128, T*18] f32: per tile [al_s[src] (8) | al_d[dst] (8) | rel | 0]."""
        T = self.T
        z = np.zeros((T, P, 18), dtype=np.float32)
        z[:, :, 0:8] = als[self.src2d[core]]
        z[:, :, 8:16] = ald[self.dst2d[core]]
        z[:, :, 16] = self.rel2d[core]
        return np.ascontiguousarray(z.transpose(1, 0, 2)).reshape(P, T * 18)

    def stream_zs2(self, als, ald, core):
        """[128, T*4] f32: per tile [al_s[src], al_d[dst], rel, 0]."""
        T = self.T
        z = np.zeros((T, P, 4), dtype=np.float32)
        z[:, :, 0] = als[self.src2d[core]]
        z[:, :, 1] = ald[self.dst2d[core]]
        z[:, :, 2] = self.rel2d[core]
        return np.ascontiguousarray(z.transpose(1, 0, 2)).reshape(P, T * 4)


# ------------------------------------------------------------------ builders

def _build_node(SH, c_in, m_h, m_al, elu, bias_in, bench_loop=1):
    """Per-node transform: hT = (elu?(xT+b)) @ w, alT = same @ wal.
    xT [c_in, SH] fp16 -> hT [m_h, SH] fp16, alT [m_al, SH] f32."""
    nc = bass.Bass()
    xT = nc.dram_tensor("xT", [c_in, SH], F16, kind="ExternalInput")
    w = nc.dram_tensor("w", [c_in, m_h], F16, kind="ExternalInput")
    wal = nc.dram_tensor("wal", [c_in, m_al], F16, kind="ExternalInput")
    if bias_in:
        bvec = nc.dram_tensor("bvec", [c_in, 1], F32, kind="ExternalInput")
    hT = nc.dram_tensor("hT", [m_h, SH], F16, kind="ExternalOutput")
    alT = nc.dram_tensor("alT", [m_al, SH], F32, kind="ExternalOutput")

    with tile.TileContext(nc) as tc:
        with (
            tc.tile_pool(name="const", bufs=1) as constp,
            tc.tile_pool(name="xs", bufs=3) as xsp,
            tc.tile_pool(name="work", bufs=3) as workp,
            tc.tile_pool(name="out", bufs=3) as outp,
            tc.tile_pool(name="psH", bufs=2, space="PSUM") as psH,
            tc.tile_pool(name="psA", bufs=2, space="PSUM") as psA,
        ):
            w_sb = constp.tile([c_in, m_h], F16)
            nc.sync.dma_start(out=w_sb[:], in_=w[:])
            wal_sb = constp.tile([c_in, m_al], F16)
            nc.sync.dma_start(out=wal_sb[:], in_=wal[:])
            if bias_in:
                b_sb = constp.tile([c_in, 1], F32)
                nc.sync.dma_start(out=b_sb[:], in_=bvec[:])

            def body(_iv=None):
                for c0 in range(0, SH, CH):
                    nb = min(CH, SH - c0)
                    xc = xsp.tile([c_in, CH], F16, tag="xc")
                    nc.sync.dma_start(out=xc[:, :nb], in_=xT[:, c0:c0 + nb])
                    rhs = xc
                    if elu:
                        if bias_in:
                            nc.vector.tensor_scalar(
                                xc[:, :nb], xc[:, :nb], b_sb[:, 0:1], None,
                                OP.add)
                        mn = workp.tile([c_in, CH], F16, tag="mn")
                        nc.vector.tensor_scalar(
                            mn[:, :nb], xc[:, :nb], 0.0, None, OP.min)
                        nc.scalar.activation(mn[:, :nb], mn[:, :nb], AF.Exp)
                        mx = workp.tile([c_in, CH], F16, tag="mx")
                        nc.vector.tensor_scalar(
                            mx[:, :nb], xc[:, :nb], 0.0, -1.0, OP.max, OP.add)
                        xe = workp.tile([c_in, CH], F16, tag="xe")
                        nc.vector.tensor_tensor(
                            out=xe[:, :nb], in0=mx[:, :nb], in1=mn[:, :nb],
                            op=OP.add)
                        rhs = xe
                    ph = psH.tile([m_h, CH], F32, tag="ph")
                    nc.tensor.matmul(ph[:, :nb], w_sb[:], rhs[:, :nb],
                                     start=True, stop=True)
                    pa = psA.tile([m_al, CH], F32, tag="pa")
                    nc.tensor.matmul(pa[:, :nb], wal_sb[:], rhs[:, :nb],
                                     start=True, stop=True)
                    h_sb = outp.tile([m_h, CH], F16, tag="h")
                    nc.scalar.activation(h_sb[:, :nb], ph[:, :nb], AF.Copy)
                    a_sb = outp.tile([m_al, CH], F32, tag="a")
                    nc.vector.tensor_copy(a_sb[:, :nb], pa[:, :nb])
                    nc.sync.dma_start(out=hT[:, c0:c0 + nb], in_=h_sb[:, :nb])
                    nc.sync.dma_start(out=alT[:, c0:c0 + nb], in_=a_sb[:, :nb])

            if bench_loop > 1:
                with tc.For_i(0, bench_loop, 1) as _iv:
                    body(_iv)
            else:
                body()
    _finalize_kernel(nc)
    return nc


def _tile_windows(T, PC, wpc):
    tile_win = []
    for i in range(wpc):
        tile_win += [i] * int(PC[i])
    first_of_win, last_of_win = {}, {}
    for t, w in enumerate(tile_win):
        first_of_win.setdefault(w, t)
        last_of_win[w] = t
    return tile_win, first_of_win, last_of_win


def _build_edge1(T, PC, wpc, bench_loop=1):
    """Layer-1 edge aggregation, 8 heads x 16ch. Streams h1[src] and the
    logit pairs; one matmul per 128-edge tile accumulates [msg | exp] into
    the window's PSUM slot. Output is the PRE-ELU aggregated feature."""
    HC, H, ZS, SLOT = 128, 8, 18, 136
    nc = bass.Bass()
    hsrc = nc.dram_tensor("hsrc", [P, T * HC], F16, kind="ExternalInput")
    zs = nc.dram_tensor("zs", [P, T * ZS], F32, kind="ExternalInput")
    iota_c = nc.dram_tensor("iota", [P, P], F16, kind="ExternalInput")
    out = nc.dram_tensor("out", [wpc * P, HC], F16, kind="ExternalOutput")

    n_groups = (T + GRP - 1) // GRP
    tile_win, first_of_win, last_of_win = _tile_windows(T, PC, wpc)

    with tile.TileContext(nc) as tc:
        with (
            tc.tile_pool(name="const", bufs=1) as constp,
            tc.tile_pool(name="zs", bufs=3) as zsp,
            tc.tile_pool(name="hs", bufs=3) as hsp,
            tc.tile_pool(name="zp", bufs=2) as zpp,
            tc.tile_pool(name="ex", bufs=2) as exp_,
            tc.tile_pool(name="msg", bufs=3) as msgp,
            tc.tile_pool(name="sel", bufs=4) as selp,
            tc.tile_pool(name="epi", bufs=2) as epip,
            tc.tile_pool(name="psW", bufs=2, space="PSUM") as psW,
        ):
            iota_sb = constp.tile([P, P], F16)
            nc.sync.dma_start(out=iota_sb[:], in_=iota_c[:])
            ebias_sb = constp.tile([P, 1], F32)
            nc.vector.memset(ebias_sb[:], EXP_BIAS)

            def edge_phase(_iv=None):
                psw = None
                for g in range(n_groups):
                    tlo, thi = g * GRP, min(T, g * GRP + GRP)
                    ng = thi - tlo
                    zs_g = zsp.tile([P, GRP * ZS], F32, tag="zs")
                    nc.sync.dma_start(out=zs_g[:, :ng * ZS],
                                      in_=zs[:, tlo * ZS:thi * ZS])
                    hs_g = hsp.tile([P, GRP * HC], F16, tag="hs")
                    nc.sync.dma_start(out=hs_g[:, :ng * HC],
                                      in_=hsrc[:, tlo * HC:thi * HC])

                    zs_r = zs_g[:].rearrange("p (t z) -> p t z", t=GRP)
                    zp_g = zpp.tile([P, GRP * H], F16, tag="zp")
                    zp_r = zp_g[:].rearrange("p (t h) -> p t h", t=GRP)
                    nc.vector.tensor_tensor(
                        out=zp_r[:, :ng, :], in0=zs_r[:, :ng, 0:8],
                        in1=zs_r[:, :ng, 8:16], op=OP.add)
                    nc.scalar.activation(zp_g[:, :ng * H], zp_g[:, :ng * H],
                                         AF.Prelu, alpha=NEG_SLOPE)

                    # bcast AP of zp over the 16 channels of each head
                    zb = zp_r[:, :ng, :]
                    zp_b = bass.AP(zb.tensor, zb.offset,
                                   [zb.ap[0], zb.ap[1], zb.ap[2], [0, 16]])
                    ex_g = exp_.tile([P, GRP * HC], F16, tag="ex")
                    ex_r = ex_g[:].rearrange("p (t h c) -> p t h c", t=GRP, h=H)
                    nc.scalar.activation(ex_r[:, :ng], zp_b, AF.Exp,
                                         bias=ebias_sb[:])

                    msg_g = msgp.tile([P, GRP * SLOT], F16, tag="msg")
                    msg_r = msg_g[:].rearrange("p (t f) -> p t f", t=GRP)
                    # denominator columns: exp written compact after the msg
                    nc.scalar.activation(msg_r[:, :ng, HC:HC + H],
                                         zp_r[:, :ng, :], AF.Exp,
                                         bias=ebias_sb[:])
                    hs_r = hs_g[:].rearrange("p (t c) -> p t c", t=GRP)
                    ex_r2 = ex_g[:].rearrange("p (t c) -> p t c", t=GRP)
                    nc.vector.tensor_tensor(
                        out=msg_r[:, :ng, 0:HC], in0=hs_r[:, :ng, :],
                        in1=ex_r2[:, :ng, :], op=OP.mult)

                    for j, t in enumerate(range(tlo, thi)):
                        w = tile_win[t]
                        S_sb = selp.tile([P, P], F16, tag="S")
                        nc.vector.tensor_scalar(
                            S_sb[:], iota_sb[:],
                            zs_g[:, j * ZS + 16:j * ZS + 17], None, OP.is_equal)
                        if t == first_of_win[w]:
                            psw = psW.tile([P, SLOT], F32, tag="psw")
                        nc.tensor.matmul(
                            psw[:], S_sb[:], msg_g[:, j * SLOT:(j + 1) * SLOT],
                            start=(t == first_of_win[w]),
                            stop=(t == last_of_win[w]))
                        if t == last_of_win[w]:
                            den = epip.tile([P, H], F32, tag="den")
                            nc.vector.tensor_scalar(
                                den[:], psw[:, HC:HC + H], EPS, None, OP.add)
                            rec = epip.tile([P, H], F32, tag="rec")
                            nc.vector.reciprocal(rec[:], den[:])
                            r_ap = rec[:]
                            r_b = bass.AP(r_ap.tensor, r_ap.offset,
                                          [r_ap.ap[0], [1, H], [0, 16]])
                            o1 = epip.tile([P, HC], F16, tag="o1")
                            nc.vector.tensor_tensor(
                                out=o1[:], in0=psw[:, 0:HC], in1=r_b,
                                op=OP.mult)
                            nc.sync.dma_start(
                                out=out[w * P:(w + 1) * P, :], in_=o1[:])

            if bench_loop > 1:
                with tc.For_i(0, bench_loop, 1) as _iv:
                    edge_phase(_iv)
            else:
                edge_phase()
    _finalize_kernel(nc)
    return nc


def _build_edge2(T, PC, wpc, bias_out, bench_loop=1):
    """Layer-2 edge aggregation, 1 head x 64ch. exp(z) is folded into the
    selection matrix (dual-op tensor_scalar), so the per-tile work is one
    TS + one matmul; the denominator rides on the stream's ones column."""
    C, CW, ZS = 64, 65, 4
    nc = bass.Bass()
    hsrc = nc.dram_tensor("hsrc", [P, T * CW], F16, kind="ExternalInput")
    zs = nc.dram_tensor("zs", [P, T * ZS], F32, kind="ExternalInput")
    iota_c = nc.dram_tensor("iota", [P, P], F16, kind="ExternalInput")
    if bias_out:
        brep = nc.dram_tensor("brep", [P, C], F32, kind="ExternalInput")
    out = nc.dram_tensor("out", [wpc * P, C], F32, kind="ExternalOutput")

    n_groups = (T + GRP - 1) // GRP
    tile_win, first_of_win, last_of_win = _tile_windows(T, PC, wpc)

    with tile.TileContext(nc) as tc:
        with (
            tc.tile_pool(name="const", bufs=1) as constp,
            tc.tile_pool(name="zs", bufs=3) as zsp,
            tc.tile_pool(name="hs", bufs=3) as hsp,
            tc.tile_pool(name="zp", bufs=2) as zpp,
            tc.tile_pool(name="sel", bufs=4) as selp,
            tc.tile_pool(name="epi", bufs=2) as epip,
            tc.tile_pool(name="psW", bufs=2, space="PSUM") as psW,
        ):
            iota_sb = constp.tile([P, P], F16)
            nc.sync.dma_start(out=iota_sb[:], in_=iota_c[:])
            ebias_sb = constp.tile([P, 1], F32)
            nc.vector.memset(ebias_sb[:], EXP_BIAS)
            if bias_out:
                brep_sb = constp.tile([P, C], F32)
                nc.sync.dma_start(out=brep_sb[:], in_=brep[:])

            def edge_phase(_iv=None):
                psw = None
                for g in range(n_groups):
                    tlo, thi = g * GRP, min(T, g * GRP + GRP)
                    ng = thi - tlo
                    zs_g = zsp.tile([P, GRP * ZS], F32, tag="zs")
                    nc.sync.dma_start(out=zs_g[:, :ng * ZS],
                                      in_=zs[:, tlo * ZS:thi * ZS])
                    hs_g = hsp.tile([P, GRP * CW], F16, tag="hs")
                    nc.sync.dma_start(out=hs_g[:, :ng * CW],
                                      in_=hsrc[:, tlo * CW:thi * CW])

                    zs_r = zs_g[:].rearrange("p (t z) -> p t z", t=GRP)
                    zp_g = zpp.tile([P, GRP], F32, tag="zp")
                    zp_r = zp_g[:].rearrange("p (t z) -> p t z", z=1)
                    nc.vector.tensor_tensor(
                        out=zp_r[:, :ng], in0=zs_r[:, :ng, 0:1],
                        in1=zs_r[:, :ng, 1:2], op=OP.add)
                    nc.scalar.activation(zp_g[:, :ng], zp_g[:, :ng],
                                         AF.Prelu, alpha=NEG_SLOPE)
                    nc.scalar.activation(zp_g[:, :ng], zp_g[:, :ng], AF.Exp,
                                         bias=ebias_sb[:])

                    for j, t in enumerate(range(tlo, thi)):
                        w = tile_win[t]
                        S_sb = selp.tile([P, P], F16, tag="S")
                        nc.vector.tensor_scalar(
                            S_sb[:], iota_sb[:],
                            zs_g[:, j * ZS + 2:j * ZS + 3],
                            zp_g[:, j:j + 1], OP.is_equal, OP.mult)
                        if t == first_of_win[w]:
                            psw = psW.tile([P, CW], F32, tag="psw")
                        nc.tensor.matmul(
                            psw[:], S_sb[:], hs_g[:, j * CW:(j + 1) * CW],
                            start=(t == first_of_win[w]),
                            stop=(t == last_of_win[w]))
                        if t == last_of_win[w]:
                            den = epip.tile([P, 1], F32, tag="den")
                            nc.vector.tensor_scalar(
                                den[:], psw[:, C:C + 1], EPS, None, OP.add)
                            rec = epip.tile([P, 1], F32, tag="rec")
                            nc.vector.reciprocal(rec[:], den[:])
                            r_ap = rec[:]
                            r_b = bass.AP(r_ap.tensor, r_ap.offset,
                                          [r_ap.ap[0], [0, C]])
                            o2 = epip.tile([P, C], F32, tag="o2")
                            nc.vector.tensor_tensor(
                                out=o2[:], in0=psw[:, 0:C], in1=r_b,
                                op=OP.mult)
                            if bias_out:
                                nc.vector.tensor_tensor(
                                    out=o2[:], in0=o2[:], in1=brep_sb[:],
                                    op=OP.add)
                            nc.sync.dma_start(
                                out=out[w * P:(w + 1) * P, :], in_=o2[:])

            if bench_loop > 1:
                with tc.For_i(0, bench_loop, 1) as _iv:
                    edge_phase(_iv)
            else:
                edge_phase()
    _finalize_kernel(nc)
    return nc


# ------------------------------------------------------------------ runner

def _fold_att(W, a):
    heads, hid = a.shape
    return np.einsum("ihc,hc->ih", W.reshape(W.shape[0], heads, hid), a)


class _GatRunner:
    def __init__(self, n_cores=N_CORES):
        self.C = n_cores
        self._graph = None
        self._graph_key = None
        self._kernels = {}
        self.last_maps = {}

    def graph(self, edge_index, n_nodes):
        key = hash(np.asarray(edge_index).tobytes())
        if key != self._graph_key:
            self._graph = _Graph(edge_index, n_nodes, self.C)
            self._graph_key = key
            self._kernels.clear()
        return self._graph

    def kernel(self, name, bench_loop=1, **kw):
        key = (name, bench_loop, tuple(sorted(kw.items())))
        if key not in self._kernels:
            g = self._graph
            if name.startswith("P"):
                self._kernels[key] = _build_node(
                    g.shard_nodes, bench_loop=bench_loop, **kw)
            elif name == "E1":
                self._kernels[key] = _build_edge1(
                    g.T, g.PC, g.wpc, bench_loop=bench_loop)
            else:
                self._kernels[key] = _build_edge2(
                    g.T, g.PC, g.wpc, bench_loop=bench_loop, **kw)
        return self._kernels[key]

    def _run(self, name, nc, maps):
        self.last_maps[name] = maps
        res = run_bass_kernel_spmd(nc, maps, core_ids=list(range(self.C)))
        return res.results

    def run(self, x, edge_index, W1, a_src1, a_dst1, b1, W2, a_src2, a_dst2,
            b2):
        C = self.C
        N, IN_C = x.shape
        HEADS, HID = a_src1.shape
        HC = HEADS * HID
        OUT_C = W2.shape[1]
        g = self.graph(edge_index, N)
        SH = g.shard_nodes
        iota_v = np.tile(np.arange(P, dtype=np.float16), (P, 1))

        # ---- P0: per-node h1 / logits --------------------------------
        xT_pad = np.zeros((IN_C, g.n_pad), dtype=np.float16)
        xT_pad[:, :N] = np.asarray(x, np.float32).T
        w1 = np.asarray(W1, np.float32)
        wal1 = np.concatenate(
            [_fold_att(w1, np.asarray(a_src1, np.float32)),
             _fold_att(w1, np.asarray(a_dst1, np.float32))], axis=1)
        mapsP0 = [{"xT": np.ascontiguousarray(xT_pad[:, k * SH:(k + 1) * SH]),
                   "w": w1.astype(np.float16),
                   "wal": wal1.astype(np.float16)} for k in range(C)]
        ncP0 = self.kernel("P0", c_in=IN_C, m_h=HC, m_al=2 * HEADS,
                           elu=False, bias_in=False)
        resP0 = self._run("P0", ncP0, mapsP0)
        h1 = np.ascontiguousarray(
            np.concatenate([r["hT"] for r in resP0], axis=1).T)  # [Np, HC] f16
        al1 = np.concatenate([r["alT"] for r in resP0], axis=1)  # [16, Np] f32
        als1 = np.ascontiguousarray(al1[:HEADS].T)
        ald1 = np.ascontiguousarray(al1[HEADS:].T)

        # ---- E1: layer-1 edge aggregation ----------------------------
        mapsE1 = [{"hsrc": g.stream_feat(h1, k),
                   "zs": g.stream_zs(als1, ald1, k),
                   "iota": iota_v} for k in range(C)]
        ncE1 = self.kernel("E1")
        resE1 = self._run("E1", ncE1, mapsE1)
        out1 = np.concatenate([r["out"] for r in resE1], axis=0)  # [Np, HC]

        # ---- P2: ELU + per-node h2 / logits --------------------------
        o1T = np.ascontiguousarray(out1.T)  # [HC, Np] f16
        w2 = np.asarray(W2, np.float32)
        wal2 = np.concatenate(
            [_fold_att(w2, np.asarray(a_src2, np.float32)),
             _fold_att(w2, np.asarray(a_dst2, np.float32))], axis=1)
        b1nz = bool(np.any(np.asarray(b1)))
        mapsP2 = []
        for k in range(C):
            m = {"xT": np.ascontiguousarray(o1T[:, k * SH:(k + 1) * SH]),
                 "w": w2.astype(np.float16),
                 "wal": wal2.astype(np.float16)}
            if b1nz:
                m["bvec"] = np.asarray(b1, np.float32).reshape(HC, 1)
            mapsP2.append(m)
        ncP2 = self.kernel("P2", c_in=HC, m_h=OUT_C, m_al=2, elu=True,
                           bias_in=b1nz)
        resP2 = self._run("P2", ncP2, mapsP2)
        h2 = np.ascontiguousarray(
            np.concatenate([r["hT"] for r in resP2], axis=1).T)  # [Np, 64]
        al2 = np.concatenate([r["alT"] for r in resP2], axis=1)
        als2, ald2 = al2[0], al2[1]

        # ---- E2: layer-2 edge aggregation ----------------------------
        b2nz = bool(np.any(np.asarray(b2)))
        mapsE2 = []
        for k in range(C):
            m = {"hsrc": g.stream_feat(h2, k, ones_col=True),
                 "zs": g.stream_zs2(als2, ald2, k),
                 "iota": iota_v}
            if b2nz:
                m["brep"] = np.tile(np.asarray(b2, np.float32), (P, 1))
            mapsE2.append(m)
        ncE2 = self.kernel("E2", bias_out=b2nz)
        resE2 = self._run("E2", ncE2, mapsE2)
        out2 = np.concatenate([r["out"] for r in resE2], axis=0)
        return out2[:N]


_RUNNER = _GatRunner()


def kernel(x, edge_index, W1, a_src1, a_dst1, b1, W2, a_src2, a_dst2, b2):
    """Full-input / full-output entry point. Returns [N, OUT_C] float32."""
    args = [np.asarray(v) for v in
            (x, edge_index, W1, a_src1, a_dst1, b1, W2, a_src2, a_dst2, b2)]
    return _RUNNER.run(*args).astype(np.float32)


# revision 11
# speedup vs baseline: 1.2211x; 1.2211x over previous
"""Trainium (trn2) Bass kernel for a 2-layer GAT over N=100k nodes / E=1.7M edges.

Strategy (v2 — node-transform / edge-aggregate split)
-----------------------------------------------------
Edges are sorted by destination on the host (index-only preprocessing); the
destination axis is sharded across the 8 NeuronCores in contiguous 128-node
windows (98 windows per core).  Each GAT layer runs as TWO SPMD kernels with
host-side index gathers (pure permutations / casts — no host FLOPs) between
them:

* node kernel (P):  h = x @ W and the folded attention logits
  al_s = x @ (W a_s), al_d = x @ (W a_d) are computed ONCE PER NODE
  (dense matmuls, ~25 us/core).  For layer 2 the ELU of the layer-1
  output is fused into this kernel's input stream.
* host: gathers per-edge streams h[src], al_s[src], al_d[dst] into the
  dst-sorted slot order (numpy fancy indexing = permutation only).
* edge kernel (E):  per 128-edge tile, z = al_s+al_d (DVE), leaky_relu &
  exp on the Scalar engine (constant -4 bias keeps fp16 exp in range and
  cancels in the softmax), messages m = h_src * exp(z) (DVE, with the
  exp broadcast pre-expanded by a Scalar-engine copy so the multiply
  runs in 2x mode), and a single matmul per tile accumulates both the
  numerator segment-sum and the denominators into one PSUM slot via an
  on-chip selection matrix S[e,n] = (rel_dst[e]==n) built with one
  tensor_scalar(is_equal).  Layer 2 has 1 head, so exp(z) is folded
  directly into S by a dual-op tensor_scalar (is_equal, mult) and the
  message multiply disappears; the denominator rides on a host-appended
  ones column of the feature stream.

vs the v1 kernel this removes the per-edge recompute of x[src] @ W (17x the
node-phase FLOPs), the streamed one-hot S^T matrix (54 MB/core of HBM reads)
and 3 of the 4 per-tile matmuls; the edge kernels are Vector-engine bound at
~1 matmul + ~1.5 DVE ops per 128-edge tile.

Environment workarounds: this container's walrus build allows only ONE
semaphore wait per instruction (split onto nop carriers post-scheduling), and
the GPSIMD ucode libraries are absent (so no dma_gather/indirect-DMA fast
paths - hence the host-gather design).
"""
import numpy as np

import concourse.bass as bass
import concourse.mybir as mybir
import concourse.tile as tile
from concourse.bass_utils import run_bass_kernel_spmd

P = 128
F16 = mybir.dt.float16
F32 = mybir.dt.float32
F8 = mybir.dt.float8e4
AF = mybir.ActivationFunctionType
OP = mybir.AluOpType
NEG_SLOPE = 0.2
EXP_BIAS = -4.0     # exp(z + EXP_BIAS): constant shift cancels in softmax
GRP = 16            # tiles per stream group
PAD_REL = 255.0     # rel value for pad slots -> is_equal never matches
N_CORES = 8
EPS = 1e-30
CH = 512            # node-kernel chunk (one PSUM bank of fp32)

# ------------------------------------------------------------------ patches

_wsplit_counter = [0]


def _split_excess_waits(nc, max_waits=1):
    """This walrus build rejects >1 sem-wait per instruction ("Too many sync
    wait commands"). Move overflow waits onto same-engine nop carriers."""
    n_split = 0
    for f in nc.m.functions:
        for blk in f.blocks:
            changed = False
            out = []
            for inst in blk.instructions:
                si = inst.sync_info
                if si is not None and len(si.on_wait) > max_waits:
                    waits = list(si.on_wait)
                    keep = waits[len(waits) - max_waits:]
                    overflow = waits[: len(waits) - max_waits]
                    for i in range(0, len(overflow), max_waits):
                        _wsplit_counter[0] += 1
                        nop = mybir.InstNoOp(
                            name=f"I-wsplit-{_wsplit_counter[0]}", ins=[], outs=[])
                        nop.engine = inst.engine
                        nop.sync_info = mybir.SyncInfo(
                            on_wait=overflow[i: i + max_waits], on_update=[])
                        out.append(nop)
                    inst.sync_info = mybir.SyncInfo(
                        on_wait=keep, on_update=list(si.on_update))
                    changed = True
                    n_split += 1
                out.append(inst)
            if changed:
                blk.instructions = out
    return n_split


def _finalize_kernel(nc):
    import bass_rust as _bass_rust
    from concourse.library_config import all_libraries, standard
    from concourse.library_overlay import lower_extended_insts

    inst_type_to_lib_mask = {}
    for lib in all_libraries:
        for inst_type in lib.instructions:
            inst_type_to_lib_mask[inst_type] = inst_type_to_lib_mask.get(
                inst_type, 0) | (1 << lib.index)
    _bass_rust.insert_library_loads(
        nc, inst_type_to_lib_mask, len(all_libraries), standard.index)
    lower_extended_insts(nc)
    _split_excess_waits(nc)


# ------------------------------------------------------------------ host prep

class _Graph:
    """Host-side index preprocessing: sort by dst, shard dst windows across
    cores, pad per-window tile counts to a global schedule so all cores run
    one identical SPMD program."""

    def __init__(self, edge_index, n_nodes, n_cores):
        self.N = n_nodes
        self.C = n_cores
        src = np.asarray(edge_index[0], dtype=np.int64)
        dst = np.asarray(edge_index[1], dtype=np.int64)
        perm = np.argsort(dst, kind="stable")
        self.src_s = src[perm].astype(np.int32)
        self.dst_s = dst[perm].astype(np.int32)

        n_win_total = (n_nodes + P - 1) // P
        self.wpc = (n_win_total + n_cores - 1) // n_cores
        self.n_win = self.wpc * n_cores
        self.shard_nodes = self.wpc * P
        self.n_pad = self.n_win * P

        bounds = np.searchsorted(self.dst_s, np.arange(0, self.n_win + 1) * P)
        counts = np.zeros((n_cores, self.wpc), dtype=np.int64)
        for k in range(n_cores):
            for i in range(self.wpc):
                w = k * self.wpc + i
                if w < n_win_total:
                    counts[k, i] = bounds[w + 1] - bounds[w]
        self.PC = np.maximum(np.ceil(counts / P).astype(np.int64).max(axis=0), 1)
        self.T = int(self.PC.sum())

        self.slot_src = np.zeros((n_cores, self.T * P), dtype=np.int32)
        self.slot_dst = np.zeros((n_cores, self.T * P), dtype=np.int32)
        self.slot_rel = np.full((n_cores, self.T * P), int(PAD_REL), dtype=np.int32)
        for k in range(n_cores):
            t0 = 0
            for i in range(self.wpc):
                w = k * self.wpc + i
                cnt = int(counts[k, i])
                if cnt > 0:
                    e0 = bounds[w]
                    sl = t0 * P
                    self.slot_src[k, sl:sl + cnt] = self.src_s[e0:e0 + cnt]
                    self.slot_dst[k, sl:sl + cnt] = self.dst_s[e0:e0 + cnt]
                    self.slot_rel[k, sl:sl + cnt] = self.dst_s[e0:e0 + cnt] - w * P
                t0 += int(self.PC[i])
        self.src2d = self.slot_src.reshape(n_cores, self.T, P)
        self.dst2d = self.slot_dst.reshape(n_cores, self.T, P)
        self.rel2d = self.slot_rel.reshape(n_cores, self.T, P)

    def stream_feat(self, table, core, ones_col=False):
        """[128, T*C] (or T*(C+1) with a trailing ones column per tile):
        col t*C+c of partition e = table[src[slot t,e], c]."""
        T, C = self.T, table.shape[1]
        W = C + 1 if ones_col else C
        out = np.empty((T, P, W), dtype=np.float16)
        out[:, :, :C] = table[self.src2d[core]]
        if ones_col:
            out[:, :, C] = 1.0
        return np.ascontiguousarray(out.transpose(1, 0, 2)).reshape(P, T * W)

    def stream_zs(self, als, ald, core):
        """[128, T*16] f16: per tile [al_s[src] (8) | al_d[dst] (8)]."""
        T = self.T
        z = np.empty((T, P, 16), dtype=np.float16)
        z[:, :, 0:8] = als[self.src2d[core]]
        z[:, :, 8:16] = ald[self.dst2d[core]]
        return np.ascontiguousarray(z.transpose(1, 0, 2)).reshape(P, T * 16)

    def stream_zs2(self, als, ald, core):
        """[128, T*2] f16: per tile [al_s[src], al_d[dst]]."""
        T = self.T
        z = np.empty((T, P, 2), dtype=np.float16)
        z[:, :, 0] = als[self.src2d[core]]
        z[:, :, 1] = ald[self.dst2d[core]]
        return np.ascontiguousarray(z.transpose(1, 0, 2)).reshape(P, T * 2)

    def stream_sel(self, core):
        """[128, T*128] fp8e4m3 one-hot: col t*128+n of partition e is
        1.0 iff rel[t,e] == n. Graph-only; shared by both layers."""
        if not hasattr(self, "_sel"):
            self._sel = {}
        if core not in self._sel:
            import ml_dtypes
            one = np.float32(1.0).astype(ml_dtypes.float8_e4m3).view(np.uint8)
            T = self.T
            arr = np.zeros((T, P, P), dtype=np.uint8)
            rel = self.rel2d[core]
            t_i, e_i = np.nonzero(rel < P)
            arr[t_i, e_i, rel[t_i, e_i]] = one
            self._sel[core] = np.ascontiguousarray(
                arr.transpose(1, 0, 2)).reshape(P, T * P).view(
                    ml_dtypes.float8_e4m3)
        return self._sel[core]


# ------------------------------------------------------------------ builders

def _build_node(SH, c_in, m_h, m_al, elu, bias_in, bench_loop=1):
    """Per-node transform: hT = (elu?(xT+b)) @ w, alT = same @ wal.
    xT [c_in, SH] fp16 -> hT [m_h, SH] fp16, alT [m_al, SH] f32."""
    nc = bass.Bass()
    xT = nc.dram_tensor("xT", [c_in, SH], F16, kind="ExternalInput")
    w = nc.dram_tensor("w", [c_in, m_h], F16, kind="ExternalInput")
    wal = nc.dram_tensor("wal", [c_in, m_al], F16, kind="ExternalInput")
    if bias_in:
        bvec = nc.dram_tensor("bvec", [c_in, 1], F32, kind="ExternalInput")
    hT = nc.dram_tensor("hT", [m_h, SH], F16, kind="ExternalOutput")
    alT = nc.dram_tensor("alT", [m_al, SH], F32, kind="ExternalOutput")

    with tile.TileContext(nc) as tc:
        with (
            tc.tile_pool(name="const", bufs=1) as constp,
            tc.tile_pool(name="xs", bufs=3) as xsp,
            tc.tile_pool(name="work", bufs=3) as workp,
            tc.tile_pool(name="out", bufs=3) as outp,
            tc.tile_pool(name="psH", bufs=2, space="PSUM") as psH,
            tc.tile_pool(name="psA", bufs=2, space="PSUM") as psA,
        ):
            w_sb = constp.tile([c_in, m_h], F16)
            nc.sync.dma_start(out=w_sb[:], in_=w[:])
            wal_sb = constp.tile([c_in, m_al], F16)
            nc.sync.dma_start(out=wal_sb[:], in_=wal[:])
            if bias_in:
                b_sb = constp.tile([c_in, 1], F32)
                nc.sync.dma_start(out=b_sb[:], in_=bvec[:])

            def body(_iv=None):
                for c0 in range(0, SH, CH):
                    nb = min(CH, SH - c0)
                    xc = xsp.tile([c_in, CH], F16, tag="xc")
                    nc.sync.dma_start(out=xc[:, :nb], in_=xT[:, c0:c0 + nb])
                    rhs = xc
                    if elu:
                        if bias_in:
                            nc.vector.tensor_scalar(
                                xc[:, :nb], xc[:, :nb], b_sb[:, 0:1], None,
                                OP.add)
                        mn = workp.tile([c_in, CH], F16, tag="mn")
                        nc.vector.tensor_scalar(
                            mn[:, :nb], xc[:, :nb], 0.0, None, OP.min)
                        nc.scalar.activation(mn[:, :nb], mn[:, :nb], AF.Exp)
                        mx = workp.tile([c_in, CH], F16, tag="mx")
                        nc.vector.tensor_scalar(
                            mx[:, :nb], xc[:, :nb], 0.0, -1.0, OP.max, OP.add)
                        xe = workp.tile([c_in, CH], F16, tag="xe")
                        nc.vector.tensor_tensor(
                            out=xe[:, :nb], in0=mx[:, :nb], in1=mn[:, :nb],
                            op=OP.add)
                        rhs = xe
                    ph = psH.tile([m_h, CH], F32, tag="ph")
                    nc.tensor.matmul(ph[:, :nb], w_sb[:], rhs[:, :nb],
                                     start=True, stop=True)
                    pa = psA.tile([m_al, CH], F32, tag="pa")
                    nc.tensor.matmul(pa[:, :nb], wal_sb[:], rhs[:, :nb],
                                     start=True, stop=True)
                    h_sb = outp.tile([m_h, CH], F16, tag="h")
                    nc.scalar.activation(h_sb[:, :nb], ph[:, :nb], AF.Copy)
                    a_sb = outp.tile([m_al, CH], F32, tag="a")
                    nc.vector.tensor_copy(a_sb[:, :nb], pa[:, :nb])
                    nc.sync.dma_start(out=hT[:, c0:c0 + nb], in_=h_sb[:, :nb])
                    nc.sync.dma_start(out=alT[:, c0:c0 + nb], in_=a_sb[:, :nb])

            if bench_loop > 1:
                with tc.For_i(0, bench_loop, 1) as _iv:
                    body(_iv)
            else:
                body()
    _finalize_kernel(nc)
    return nc


def _tile_windows(T, PC, wpc):
    tile_win = []
    for i in range(wpc):
        tile_win += [i] * int(PC[i])
    first_of_win, last_of_win = {}, {}
    for t, w in enumerate(tile_win):
        first_of_win.setdefault(w, t)
        last_of_win[w] = t
    return tile_win, first_of_win, last_of_win


def _build_edge1(T, PC, wpc, bench_loop=1):
    """Layer-1 edge aggregation, 8 heads x 16ch, (c,h)-interleaved channel
    order (channel c*8+h = head h, dim c). Streams h1[src], the fp8 one-hot
    selection matrix, and the logit pairs; one mixed fp8xfp16 matmul per
    128-edge tile accumulates [msg | exp] into the window's PSUM slot.
    Output is the PRE-ELU aggregated feature in (c,h) order."""
    HC, H, ZS, SLOT = 128, 8, 16, 136
    nc = bass.Bass()
    hsrc = nc.dram_tensor("hsrc", [P, T * HC], F16, kind="ExternalInput")
    s8 = nc.dram_tensor("s8", [P, T * P], F8, kind="ExternalInput")
    zs = nc.dram_tensor("zs", [P, T * ZS], F16, kind="ExternalInput")
    out = nc.dram_tensor("out", [wpc * P, HC], F16, kind="ExternalOutput")

    n_groups = (T + GRP - 1) // GRP
    tile_win, first_of_win, last_of_win = _tile_windows(T, PC, wpc)

    with tile.TileContext(nc) as tc:
        with (
            tc.tile_pool(name="const", bufs=1) as constp,
            tc.tile_pool(name="zs", bufs=3) as zsp,
            tc.tile_pool(name="hs", bufs=3) as hsp,
            tc.tile_pool(name="s8", bufs=3) as s8p,
            tc.tile_pool(name="zp", bufs=2) as zpp,
            tc.tile_pool(name="ex", bufs=2) as exp_,
            tc.tile_pool(name="msg", bufs=3) as msgp,
            tc.tile_pool(name="epi", bufs=2) as epip,
            tc.tile_pool(name="psW", bufs=2, space="PSUM") as psW,
        ):
            ebias_sb = constp.tile([P, 1], F32)
            nc.vector.memset(ebias_sb[:], EXP_BIAS)

            def edge_phase(_iv=None):
                psw = None
                for g in range(n_groups):
                    tlo, thi = g * GRP, min(T, g * GRP + GRP)
                    ng = thi - tlo
                    zs_g = zsp.tile([P, GRP * ZS], F16, tag="zs")
                    nc.sync.dma_start(out=zs_g[:, :ng * ZS],
                                      in_=zs[:, tlo * ZS:thi * ZS])
                    hs_g = hsp.tile([P, GRP * HC], F16, tag="hs")
                    nc.sync.dma_start(out=hs_g[:, :ng * HC],
                                      in_=hsrc[:, tlo * HC:thi * HC])
                    s8_g = s8p.tile([P, GRP * P], F8, tag="s8")
                    nc.sync.dma_start(out=s8_g[:, :ng * P],
                                      in_=s8[:, tlo * P:thi * P])

                    zs_r = zs_g[:].rearrange("p (t z) -> p t z", t=GRP)
                    zp_g = zpp.tile([P, GRP * H], F16, tag="zp")
                    zp_r = zp_g[:].rearrange("p (t h) -> p t h", t=GRP)
                    nc.vector.tensor_tensor(
                        out=zp_r[:, :ng, :], in0=zs_r[:, :ng, 0:8],
                        in1=zs_r[:, :ng, 8:16], op=OP.add)
                    nc.scalar.activation(zp_g[:, :ng * H], zp_g[:, :ng * H],
                                         AF.Prelu, alpha=NEG_SLOPE)
                    ex_g = exp_.tile([P, GRP * H], F16, tag="ex")
                    nc.scalar.activation(ex_g[:, :ng * H], zp_g[:, :ng * H],
                                         AF.Exp, bias=ebias_sb[:])

                    # expand exp over the 16 dims of each head in (c,h)
                    # order AND lay the compact denominator copy at c=16,
                    # i.e. cols 128:136 - one 4x-mode DVE copy does both.
                    msg_g = msgp.tile([P, GRP * SLOT], F16, tag="msg")
                    ex_r = ex_g[:].rearrange("p (t h) -> p t h", t=GRP)
                    eb = ex_r[:, :ng, :]
                    ex_b = bass.AP(eb.tensor, eb.offset,
                                   [eb.ap[0], eb.ap[1], [0, 17], eb.ap[2]])
                    msg_r = msg_g[:].rearrange("p (t f) -> p t f", t=GRP)
                    mr = msg_r[:, :ng, :]
                    msg_chr = bass.AP(mr.tensor, mr.offset,
                                      [mr.ap[0], mr.ap[1], [8, 17], [1, 8]])
                    nc.vector.tensor_copy(msg_chr, ex_b)
                    hs_r = hs_g[:].rearrange("p (t c) -> p t c", t=GRP)
                    nc.vector.tensor_tensor(
                        out=msg_r[:, :ng, 0:HC], in0=hs_r[:, :ng, :],
                        in1=msg_r[:, :ng, 0:HC], op=OP.mult)

                    for j, t in enumerate(range(tlo, thi)):
                        w = tile_win[t]
                        if t == first_of_win[w]:
                            psw = psW.tile([P, SLOT], F32, tag="psw")
                        nc.tensor.matmul(
                            psw[:], s8_g[:, j * P:(j + 1) * P],
                            msg_g[:, j * SLOT:(j + 1) * SLOT],
                            start=(t == first_of_win[w]),
                            stop=(t == last_of_win[w]))
                        if t == last_of_win[w]:
                            den = epip.tile([P, H], F32, tag="den")
                            nc.scalar.activation(den[:], psw[:, HC:HC + H],
                                                 AF.Copy, bias=EPS)
                            rec = epip.tile([P, H], F16, tag="rec")
                            with nc.allow_low_precision(
                                    reason="softmax denominators are O(1)"):
                                nc.vector.reciprocal(rec[:], den[:])
                            o1p = epip.tile([P, HC], F16, tag="o1p")
                            nc.scalar.activation(o1p[:], psw[:, 0:HC],
                                                 AF.Copy)
                            r_ap = rec[:]
                            r_b = bass.AP(r_ap.tensor, r_ap.offset,
                                          [r_ap.ap[0], [0, 16], [1, H]])
                            o1 = epip.tile([P, HC], F16, tag="o1")
                            o1_r = o1[:].rearrange("p (c h) -> p c h", c=16)
                            o1p_r = o1p[:].rearrange("p (c h) -> p c h", c=16)
                            nc.vector.tensor_tensor(
                                out=o1_r, in0=o1p_r, in1=r_b, op=OP.mult)
                            nc.sync.dma_start(
                                out=out[w * P:(w + 1) * P, :], in_=o1[:])

            if bench_loop > 1:
                with tc.For_i(0, bench_loop, 1) as _iv:
                    edge_phase(_iv)
            else:
                edge_phase()
    _finalize_kernel(nc)
    return nc


def _build_edge2(T, PC, wpc, bias_out, bench_loop=1):
    """Layer-2 edge aggregation, 1 head x 64ch. Messages are the streamed
    h2[src] (with a host-appended ones column for the denominator) scaled
    by the broadcast exp(z); one mixed fp8xfp16 matmul per tile against the
    streamed one-hot selection matrix."""
    C, CW, ZS = 64, 65, 2
    nc = bass.Bass()
    hsrc = nc.dram_tensor("hsrc", [P, T * CW], F16, kind="ExternalInput")
    s8 = nc.dram_tensor("s8", [P, T * P], F8, kind="ExternalInput")
    zs = nc.dram_tensor("zs", [P, T * ZS], F16, kind="ExternalInput")
    if bias_out:
        brep = nc.dram_tensor("brep", [P, C], F32, kind="ExternalInput")
    out = nc.dram_tensor("out", [wpc * P, C], F32, kind="ExternalOutput")

    n_groups = (T + GRP - 1) // GRP
    tile_win, first_of_win, last_of_win = _tile_windows(T, PC, wpc)

    with tile.TileContext(nc) as tc:
        with (
            tc.tile_pool(name="const", bufs=1) as constp,
            tc.tile_pool(name="zs", bufs=3) as zsp,
            tc.tile_pool(name="hs", bufs=3) as hsp,
            tc.tile_pool(name="s8", bufs=3) as s8p,
            tc.tile_pool(name="zp", bufs=2) as zpp,
            tc.tile_pool(name="msg", bufs=3) as msgp,
            tc.tile_pool(name="epi", bufs=2) as epip,
            tc.tile_pool(name="psW", bufs=2, space="PSUM") as psW,
        ):
            ebias_sb = constp.tile([P, 1], F32)
            nc.vector.memset(ebias_sb[:], EXP_BIAS)
            if bias_out:
                brep_sb = constp.tile([P, C], F32)
                nc.sync.dma_start(out=brep_sb[:], in_=brep[:])

            def edge_phase(_iv=None):
                psw = None
                for g in range(n_groups):
                    tlo, thi = g * GRP, min(T, g * GRP + GRP)
                    ng = thi - tlo
                    zs_g = zsp.tile([P, GRP * ZS], F16, tag="zs")
                    nc.sync.dma_start(out=zs_g[:, :ng * ZS],
                                      in_=zs[:, tlo * ZS:thi * ZS])
                    hs_g = hsp.tile([P, GRP * CW], F16, tag="hs")
                    nc.sync.dma_start(out=hs_g[:, :ng * CW],
                                      in_=hsrc[:, tlo * CW:thi * CW])
                    s8_g = s8p.tile([P, GRP * P], F8, tag="s8")
                    nc.sync.dma_start(out=s8_g[:, :ng * P],
                                      in_=s8[:, tlo * P:thi * P])

                    zs_r = zs_g[:].rearrange("p (t z) -> p t z", t=GRP)
                    zp_g = zpp.tile([P, GRP], F16, tag="zp")
                    zp_r = zp_g[:].rearrange("p (t z) -> p t z", z=1)
                    nc.vector.tensor_tensor(
                        out=zp_r[:, :ng], in0=zs_r[:, :ng, 0:1],
                        in1=zs_r[:, :ng, 1:2], op=OP.add)
                    nc.scalar.activation(zp_g[:, :ng], zp_g[:, :ng],
                                         AF.Prelu, alpha=NEG_SLOPE)
                    nc.scalar.activation(zp_g[:, :ng], zp_g[:, :ng], AF.Exp,
                                         bias=ebias_sb[:])

                    # msg = h2src * exp(z) broadcast over the 65 columns
                    msg_g = msgp.tile([P, GRP * CW], F16, tag="msg")
                    msg_r = msg_g[:].rearrange("p (t c) -> p t c", t=GRP)
                    hs_r = hs_g[:].rearrange("p (t c) -> p t c", t=GRP)
                    zb = zp_r[:, :ng]
                    zp_b = bass.AP(zb.tensor, zb.offset,
                                   [zb.ap[0], zb.ap[1], [0, CW]])
                    nc.vector.tensor_tensor(
                        out=msg_r[:, :ng, :], in0=hs_r[:, :ng, :],
                        in1=zp_b, op=OP.mult)

                    for j, t in enumerate(range(tlo, thi)):
                        w = tile_win[t]
                        if t == first_of_win[w]:
                            psw = psW.tile([P, CW], F32, tag="psw")
                        nc.tensor.matmul(
                            psw[:], s8_g[:, j * P:(j + 1) * P],
                            msg_g[:, j * CW:(j + 1) * CW],
                            start=(t == first_of_win[w]),
                            stop=(t == last_of_win[w]))
                        if t == last_of_win[w]:
                            den = epip.tile([P, 1], F32, tag="den")
                            nc.scalar.activation(den[:], psw[:, C:C + 1],
                                                 AF.Copy, bias=EPS)
                            rec = epip.tile([P, 1], F32, tag="rec")
                            nc.vector.reciprocal(rec[:], den[:])
                            r_ap = rec[:]
                            r_b = bass.AP(r_ap.tensor, r_ap.offset,
                                          [r_ap.ap[0], [0, C]])
                            o2 = epip.tile([P, C], F32, tag="o2")
                            nc.vector.tensor_tensor(
                                out=o2[:], in0=psw[:, 0:C], in1=r_b,
                                op=OP.mult)
                            if bias_out:
                                nc.vector.tensor_tensor(
                                    out=o2[:], in0=o2[:], in1=brep_sb[:],
                                    op=OP.add)
                            nc.sync.dma_start(
                                out=out[w * P:(w + 1) * P, :], in_=o2[:])

            if bench_loop > 1:
                with tc.For_i(0, bench_loop, 1) as _iv:
                    edge_phase(_iv)
            else:
                edge_phase()
    _finalize_kernel(nc)
    return nc


# ------------------------------------------------------------------ runner

def _fold_att(W, a):
    heads, hid = a.shape
    return np.einsum("ihc,hc->ih", W.reshape(W.shape[0], heads, hid), a)


class _GatRunner:
    def __init__(self, n_cores=N_CORES):
        self.C = n_cores
        self._graph = None
        self._graph_key = None
        self._kernels = {}
        self.last_maps = {}

    def graph(self, edge_index, n_nodes):
        key = hash(np.asarray(edge_index).tobytes())
        if key != self._graph_key:
            self._graph = _Graph(edge_index, n_nodes, self.C)
            self._graph_key = key
            self._kernels.clear()
        return self._graph

    def kernel(self, name, bench_loop=1, **kw):
        key = (name, bench_loop, tuple(sorted(kw.items())))
        if key not in self._kernels:
            g = self._graph
            if name.startswith("P"):
                self._kernels[key] = _build_node(
                    g.shard_nodes, bench_loop=bench_loop, **kw)
            elif name == "E1":
                self._kernels[key] = _build_edge1(
                    g.T, g.PC, g.wpc, bench_loop=bench_loop)
            else:
                self._kernels[key] = _build_edge2(
                    g.T, g.PC, g.wpc, bench_loop=bench_loop, **kw)
        return self._kernels[key]

    def _run(self, name, nc, maps):
        self.last_maps[name] = maps
        res = run_bass_kernel_spmd(nc, maps, core_ids=list(range(self.C)))
        return res.results

    def run(self, x, edge_index, W1, a_src1, a_dst1, b1, W2, a_src2, a_dst2,
            b2):
        C = self.C
        N, IN_C = x.shape
        HEADS, HID = a_src1.shape
        HC = HEADS * HID
        OUT_C = W2.shape[1]
        g = self.graph(edge_index, N)
        SH = g.shard_nodes
        # (c,h)-interleaved channel order for the layer-1 hidden features:
        # col c*H+h of h1 holds math channel h*HID+c. Folded into W1's
        # columns (P0) and W2's rows (P2) on the host - pure permutation.
        perm = np.array([(j % HEADS) * HID + j // HEADS
                         for j in range(HC)], dtype=np.int64)

        # ---- P0: per-node h1 / logits --------------------------------
        xT_pad = np.zeros((IN_C, g.n_pad), dtype=np.float16)
        xT_pad[:, :N] = np.asarray(x, np.float32).T
        w1 = np.asarray(W1, np.float32)
        wal1 = np.concatenate(
            [_fold_att(w1, np.asarray(a_src1, np.float32)),
             _fold_att(w1, np.asarray(a_dst1, np.float32))], axis=1)
        mapsP0 = [{"xT": np.ascontiguousarray(xT_pad[:, k * SH:(k + 1) * SH]),
                   "w": np.ascontiguousarray(w1[:, perm]).astype(np.float16),
                   "wal": wal1.astype(np.float16)} for k in range(C)]
        ncP0 = self.kernel("P0", c_in=IN_C, m_h=HC, m_al=2 * HEADS,
                           elu=False, bias_in=False)
        resP0 = self._run("P0", ncP0, mapsP0)
        h1 = np.ascontiguousarray(
            np.concatenate([r["hT"] for r in resP0], axis=1).T)  # [Np,HC] f16
        al1 = np.concatenate([r["alT"] for r in resP0], axis=1)  # [16,Np] f32
        als1 = np.ascontiguousarray(al1[:HEADS].T.astype(np.float16))
        ald1 = np.ascontiguousarray(al1[HEADS:].T.astype(np.float16))

        # ---- E1: layer-1 edge aggregation ----------------------------
        mapsE1 = [{"hsrc": g.stream_feat(h1, k),
                   "s8": g.stream_sel(k),
                   "zs": g.stream_zs(als1, ald1, k)} for k in range(C)]
        ncE1 = self.kernel("E1")
        resE1 = self._run("E1", ncE1, mapsE1)
        out1 = np.concatenate([r["out"] for r in resE1], axis=0)  # [Np, HC]

        # ---- P2: ELU + per-node h2 / logits --------------------------
        o1T = np.ascontiguousarray(out1.T)  # [HC, Np] f16, (c,h) rows
        w2 = np.asarray(W2, np.float32)
        wal2 = np.concatenate(
            [_fold_att(w2, np.asarray(a_src2, np.float32)),
             _fold_att(w2, np.asarray(a_dst2, np.float32))], axis=1)
        b1nz = bool(np.any(np.asarray(b1)))
        mapsP2 = []
        for k in range(C):
            m = {"xT": np.ascontiguousarray(o1T[:, k * SH:(k + 1) * SH]),
                 "w": np.ascontiguousarray(w2[perm]).astype(np.float16),
                 "wal": np.ascontiguousarray(wal2[perm]).astype(np.float16)}
            if b1nz:
                m["bvec"] = np.asarray(b1, np.float32)[perm].reshape(HC, 1)
            mapsP2.append(m)
        ncP2 = self.kernel("P2", c_in=HC, m_h=OUT_C, m_al=2, elu=True,
                           bias_in=b1nz)
        resP2 = self._run("P2", ncP2, mapsP2)
        h2 = np.ascontiguousarray(
            np.concatenate([r["hT"] for r in resP2], axis=1).T)  # [Np, 64]
        al2 = np.concatenate([r["alT"] for r in resP2], axis=1)
        als2 = al2[0].astype(np.float16)
        ald2 = al2[1].astype(np.float16)

        # ---- E2: layer-2 edge aggregation ----------------------------
        b2nz = bool(np.any(np.asarray(b2)))
        mapsE2 = []
        for k in range(C):
            m = {"hsrc": g.stream_feat(h2, k, ones_col=True),
                 "s8": g.stream_sel(k),
                 "zs": g.stream_zs2(als2, ald2, k)}
            if b2nz:
                m["brep"] = np.tile(np.asarray(b2, np.float32), (P, 1))
            mapsE2.append(m)
        ncE2 = self.kernel("E2", bias_out=b2nz)
        resE2 = self._run("E2", ncE2, mapsE2)
        out2 = np.concatenate([r["out"] for r in resE2], axis=0)
        return out2[:N]


_RUNNER = _GatRunner()


def kernel(x, edge_index, W1, a_src1, a_dst1, b1, W2, a_src2, a_dst2, b2):
    """Full-input / full-output entry point. Returns [N, OUT_C] float32."""
    args = [np.asarray(v) for v in
            (x, edge_index, W1, a_src1, a_dst1, b1, W2, a_src2, a_dst2, b2)]
    return _RUNNER.run(*args).astype(np.float32)


# revision 17
# speedup vs baseline: 1.5144x; 1.2402x over previous
"""Trainium (trn2) Bass kernel for a 2-layer GAT over N=100k nodes / E=1.7M edges.

Strategy (v2 — node-transform / edge-aggregate split)
-----------------------------------------------------
Edges are sorted by destination on the host (index-only preprocessing); the
destination axis is sharded across the 8 NeuronCores in contiguous 128-node
windows (98 windows per core).  Each GAT layer runs as TWO SPMD kernels with
host-side index gathers (pure permutations / casts — no host FLOPs) between
them:

* node kernel (P):  h = x @ W and the folded attention logits
  al_s = x @ (W a_s), al_d = x @ (W a_d) are computed ONCE PER NODE
  (dense matmuls, ~25 us/core).  For layer 2 the ELU of the layer-1
  output is fused into this kernel's input stream.
* host: gathers per-edge streams h[src], al_s[src], al_d[dst] into the
  dst-sorted slot order (numpy fancy indexing = permutation only).
* edge kernel (E):  per 128-edge tile, z = al_s+al_d (DVE), leaky_relu &
  exp on the Scalar engine (constant -4 bias keeps fp16 exp in range and
  cancels in the softmax), messages m = h_src * exp(z) (DVE, with the
  exp broadcast pre-expanded by a Scalar-engine copy so the multiply
  runs in 2x mode), and a single matmul per tile accumulates both the
  numerator segment-sum and the denominators into one PSUM slot via an
  on-chip selection matrix S[e,n] = (rel_dst[e]==n) built with one
  tensor_scalar(is_equal).  Layer 2 has 1 head, so exp(z) is folded
  directly into S by a dual-op tensor_scalar (is_equal, mult) and the
  message multiply disappears; the denominator rides on a host-appended
  ones column of the feature stream.

vs the v1 kernel this removes the per-edge recompute of x[src] @ W (17x the
node-phase FLOPs), the streamed one-hot S^T matrix (54 MB/core of HBM reads)
and 3 of the 4 per-tile matmuls; the edge kernels are Vector-engine bound at
~1 matmul + ~1.5 DVE ops per 128-edge tile.

Environment workarounds: this container's walrus build allows only ONE
semaphore wait per instruction (split onto nop carriers post-scheduling), and
the GPSIMD ucode libraries are absent (so no dma_gather/indirect-DMA fast
paths - hence the host-gather design).
"""
import numpy as np

import concourse.bass as bass
import concourse.mybir as mybir
import concourse.tile as tile
from concourse.bass_utils import run_bass_kernel_spmd

P = 128
F16 = mybir.dt.float16
F32 = mybir.dt.float32
F8 = mybir.dt.float8e4
AF = mybir.ActivationFunctionType
OP = mybir.AluOpType
NEG_SLOPE = 0.2
EXP_BIAS = -4.0     # exp(z + EXP_BIAS): constant shift cancels in softmax
GRP = 16            # tiles per stream group
PAD_REL = 255.0     # rel value for pad slots -> is_equal never matches
N_CORES = 8
EPS = 1e-30
CH = 512            # node-kernel chunk (one PSUM bank of fp32)

# ------------------------------------------------------------------ patches

_wsplit_counter = [0]


def _split_excess_waits(nc, max_waits=1):
    """This walrus build rejects >1 sem-wait per instruction ("Too many sync
    wait commands"). Move overflow waits onto same-engine nop carriers."""
    n_split = 0
    for f in nc.m.functions:
        for blk in f.blocks:
            changed = False
            out = []
            for inst in blk.instructions:
                si = inst.sync_info
                if si is not None and len(si.on_wait) > max_waits:
                    waits = list(si.on_wait)
                    keep = waits[len(waits) - max_waits:]
                    overflow = waits[: len(waits) - max_waits]
                    for i in range(0, len(overflow), max_waits):
                        _wsplit_counter[0] += 1
                        nop = mybir.InstNoOp(
                            name=f"I-wsplit-{_wsplit_counter[0]}", ins=[], outs=[])
                        nop.engine = inst.engine
                        nop.sync_info = mybir.SyncInfo(
                            on_wait=overflow[i: i + max_waits], on_update=[])
                        out.append(nop)
                    inst.sync_info = mybir.SyncInfo(
                        on_wait=keep, on_update=list(si.on_update))
                    changed = True
                    n_split += 1
                out.append(inst)
            if changed:
                blk.instructions = out
    return n_split


def _finalize_kernel(nc):
    import bass_rust as _bass_rust
    from concourse.library_config import all_libraries, standard
    from concourse.library_overlay import lower_extended_insts

    inst_type_to_lib_mask = {}
    for lib in all_libraries:
        for inst_type in lib.instructions:
            inst_type_to_lib_mask[inst_type] = inst_type_to_lib_mask.get(
                inst_type, 0) | (1 << lib.index)
    _bass_rust.insert_library_loads(
        nc, inst_type_to_lib_mask, len(all_libraries), standard.index)
    lower_extended_insts(nc)
    _split_excess_waits(nc)


# ------------------------------------------------------------------ host prep

class _Graph:
    """Host-side index preprocessing: sort by dst, shard dst windows across
    cores, pad per-window tile counts to a global schedule so all cores run
    one identical SPMD program."""

    def __init__(self, edge_index, n_nodes, n_cores):
        self.N = n_nodes
        self.C = n_cores
        src = np.asarray(edge_index[0], dtype=np.int64)
        dst = np.asarray(edge_index[1], dtype=np.int64)
        perm = np.argsort(dst, kind="stable")
        self.src_s = src[perm].astype(np.int32)
        self.dst_s = dst[perm].astype(np.int32)

        n_win_total = (n_nodes + P - 1) // P
        self.wpc = (n_win_total + n_cores - 1) // n_cores
        self.n_win = self.wpc * n_cores
        self.shard_nodes = self.wpc * P
        self.n_pad = self.n_win * P

        bounds = np.searchsorted(self.dst_s, np.arange(0, self.n_win + 1) * P)
        counts = np.zeros((n_cores, self.wpc), dtype=np.int64)
        for k in range(n_cores):
            for i in range(self.wpc):
                w = k * self.wpc + i
                if w < n_win_total:
                    counts[k, i] = bounds[w + 1] - bounds[w]
        self.PC = np.maximum(np.ceil(counts / P).astype(np.int64).max(axis=0), 1)
        self.T = int(self.PC.sum())

        self.slot_src = np.zeros((n_cores, self.T * P), dtype=np.int32)
        self.slot_dst = np.zeros((n_cores, self.T * P), dtype=np.int32)
        self.slot_rel = np.full((n_cores, self.T * P), int(PAD_REL), dtype=np.int32)
        for k in range(n_cores):
            t0 = 0
            for i in range(self.wpc):
                w = k * self.wpc + i
                cnt = int(counts[k, i])
                if cnt > 0:
                    e0 = bounds[w]
                    sl = t0 * P
                    self.slot_src[k, sl:sl + cnt] = self.src_s[e0:e0 + cnt]
                    self.slot_dst[k, sl:sl + cnt] = self.dst_s[e0:e0 + cnt]
                    self.slot_rel[k, sl:sl + cnt] = self.dst_s[e0:e0 + cnt] - w * P
                t0 += int(self.PC[i])
        self.src2d = self.slot_src.reshape(n_cores, self.T, P)
        self.dst2d = self.slot_dst.reshape(n_cores, self.T, P)
        self.rel2d = self.slot_rel.reshape(n_cores, self.T, P)

    def stream_feat(self, table, core, ones_col=False):
        """[128, T*C] (or T*(C+1) with a trailing ones column per tile):
        col t*C+c of partition e = table[src[slot t,e], c]."""
        T, C = self.T, table.shape[1]
        W = C + 1 if ones_col else C
        out = np.empty((T, P, W), dtype=np.float16)
        out[:, :, :C] = table[self.src2d[core]]
        if ones_col:
            out[:, :, C] = 1.0
        return np.ascontiguousarray(out.transpose(1, 0, 2)).reshape(P, T * W)

    def stream_zs(self, als, ald, core):
        """[128, T*16] f16: per tile [al_s[src] (8) | al_d[dst] (8)]."""
        T = self.T
        z = np.empty((T, P, 16), dtype=np.float16)
        z[:, :, 0:8] = als[self.src2d[core]]
        z[:, :, 8:16] = ald[self.dst2d[core]]
        return np.ascontiguousarray(z.transpose(1, 0, 2)).reshape(P, T * 16)

    def stream_zs2(self, als, ald, core):
        """[128, T*2] f16: per tile [al_s[src], al_d[dst]]."""
        T = self.T
        z = np.empty((T, P, 2), dtype=np.float16)
        z[:, :, 0] = als[self.src2d[core]]
        z[:, :, 1] = ald[self.dst2d[core]]
        return np.ascontiguousarray(z.transpose(1, 0, 2)).reshape(P, T * 2)

    def stream_sel(self, core):
        """[128, T*128] fp8e4m3 one-hot: col t*128+n of partition e is
        1.0 iff rel[t,e] == n. Graph-only; shared by both layers."""
        if not hasattr(self, "_sel"):
            self._sel = {}
        if core not in self._sel:
            import ml_dtypes
            one = np.float32(1.0).astype(ml_dtypes.float8_e4m3).view(np.uint8)
            T = self.T
            arr = np.zeros((T, P, P), dtype=np.uint8)
            rel = self.rel2d[core]
            t_i, e_i = np.nonzero(rel < P)
            arr[t_i, e_i, rel[t_i, e_i]] = one
            self._sel[core] = np.ascontiguousarray(
                arr.transpose(1, 0, 2)).reshape(P, T * P).view(
                    ml_dtypes.float8_e4m3)
        return self._sel[core]


# ------------------------------------------------------------------ builders

def _build_node(SH, c_in, m_h, m_al, elu, bias_in, bench_loop=1):
    """Per-node transform: hT = (elu?(xT+b)) @ w, alT = same @ wal.
    When m_h+m_al <= 128 the two matmuls merge into one (w carries the
    al columns and alT is folded into hT's extra rows)."""
    merged = (m_h + m_al) <= P
    M = m_h + m_al if merged else m_h
    nc = bass.Bass()
    xT = nc.dram_tensor("xT", [c_in, SH], F16, kind="ExternalInput")
    w = nc.dram_tensor("w", [c_in, M], F16, kind="ExternalInput")
    if not merged:
        wal = nc.dram_tensor("wal", [c_in, m_al], F16, kind="ExternalInput")
    if bias_in:
        bvec = nc.dram_tensor("bvec", [c_in, 1], F32, kind="ExternalInput")
    hT = nc.dram_tensor("hT", [M, SH], F16, kind="ExternalOutput")
    if not merged:
        alT = nc.dram_tensor("alT", [m_al, SH], F16, kind="ExternalOutput")

    with tile.TileContext(nc) as tc:
        with (
            tc.tile_pool(name="const", bufs=1) as constp,
            tc.tile_pool(name="xs", bufs=4) as xsp,
            tc.tile_pool(name="work", bufs=4) as workp,
            tc.tile_pool(name="out", bufs=4) as outp,
            tc.tile_pool(name="psH", bufs=3, space="PSUM") as psH,
            tc.tile_pool(name="psA", bufs=3, space="PSUM") as psA,
        ):
            w_sb = constp.tile([c_in, M], F16)
            nc.sync.dma_start(out=w_sb[:], in_=w[:])
            if not merged:
                wal_sb = constp.tile([c_in, m_al], F16)
                nc.sync.dma_start(out=wal_sb[:], in_=wal[:])
            if bias_in:
                b_sb = constp.tile([c_in, 1], F32)
                nc.sync.dma_start(out=b_sb[:], in_=bvec[:])

            def body(_iv=None):
                for c0 in range(0, SH, CH):
                    nb = min(CH, SH - c0)
                    xc = xsp.tile([c_in, CH], F16, tag="xc")
                    nc.sync.dma_start(out=xc[:, :nb], in_=xT[:, c0:c0 + nb])
                    rhs = xc
                    if elu:
                        if bias_in:
                            nc.vector.tensor_scalar(
                                xc[:, :nb], xc[:, :nb], b_sb[:, 0:1], None,
                                OP.add)
                        mn = workp.tile([c_in, CH], F16, tag="mn")
                        nc.vector.tensor_scalar(
                            mn[:, :nb], xc[:, :nb], 0.0, None, OP.min)
                        nc.scalar.activation(mn[:, :nb], mn[:, :nb], AF.Exp)
                        mx = workp.tile([c_in, CH], F16, tag="mx")
                        nc.vector.tensor_scalar(
                            mx[:, :nb], xc[:, :nb], 0.0, -1.0, OP.max, OP.add)
                        xe = workp.tile([c_in, CH], F16, tag="xe")
                        nc.vector.tensor_tensor(
                            out=xe[:, :nb], in0=mx[:, :nb], in1=mn[:, :nb],
                            op=OP.add)
                        rhs = xe
                    ph = psH.tile([M, CH], F32, tag="ph")
                    nc.tensor.matmul(ph[:, :nb], w_sb[:], rhs[:, :nb],
                                     start=True, stop=True)
                    h_sb = outp.tile([M, CH], F16, tag="h")
                    nc.scalar.activation(h_sb[:, :nb], ph[:, :nb], AF.Copy)
                    nc.scalar.dma_start(out=hT[:, c0:c0 + nb],
                                        in_=h_sb[:, :nb])
                    if not merged:
                        pa = psA.tile([m_al, CH], F32, tag="pa")
                        nc.tensor.matmul(pa[:, :nb], wal_sb[:], rhs[:, :nb],
                                         start=True, stop=True)
                        a_sb = outp.tile([m_al, CH], F16, tag="a")
                        nc.vector.tensor_copy(a_sb[:, :nb], pa[:, :nb])
                        nc.scalar.dma_start(out=alT[:, c0:c0 + nb],
                                            in_=a_sb[:, :nb])

            if bench_loop > 1:
                with tc.For_i(0, bench_loop, 1) as _iv:
                    body(_iv)
            else:
                body()
    _finalize_kernel(nc)
    return nc


def _tile_windows(T, PC, wpc):
    tile_win = []
    for i in range(wpc):
        tile_win += [i] * int(PC[i])
    first_of_win, last_of_win = {}, {}
    for t, w in enumerate(tile_win):
        first_of_win.setdefault(w, t)
        last_of_win[w] = t
    return tile_win, first_of_win, last_of_win


def _build_edge1(T, PC, wpc, bench_loop=1):
    """Layer-1 edge aggregation, 8 heads x 16ch, (c,h)-interleaved channel
    order (channel c*8+h = head h, dim c). Streams h1[src], the fp8 one-hot
    selection matrix, and the logit pairs; one mixed fp8xfp16 matmul per
    128-edge tile accumulates [msg | exp] into the window's PSUM slot.
    Output is the PRE-ELU aggregated feature in (c,h) order."""
    HC, H, ZS, SLOT = 128, 8, 16, 136
    nc = bass.Bass()
    hsrc = nc.dram_tensor("hsrc", [P, T * HC], F16, kind="ExternalInput")
    s8 = nc.dram_tensor("s8", [P, T * P], F8, kind="ExternalInput")
    zs = nc.dram_tensor("zs", [P, T * ZS], F16, kind="ExternalInput")
    out = nc.dram_tensor("out", [wpc * P, HC], F16, kind="ExternalOutput")

    n_groups = (T + GRP - 1) // GRP
    tile_win, first_of_win, last_of_win = _tile_windows(T, PC, wpc)

    with tile.TileContext(nc) as tc:
        with (
            tc.tile_pool(name="const", bufs=1) as constp,
            tc.tile_pool(name="zs", bufs=4) as zsp,
            tc.tile_pool(name="hs", bufs=4) as hsp,
            tc.tile_pool(name="s8", bufs=4) as s8p,
            tc.tile_pool(name="zp", bufs=3) as zpp,
            tc.tile_pool(name="msg", bufs=4) as msgp,
            tc.tile_pool(name="epi", bufs=3) as epip,
            tc.tile_pool(name="psW", bufs=3, space="PSUM") as psW,
        ):
            ebias_sb = constp.tile([P, 1], F32)
            nc.vector.memset(ebias_sb[:], EXP_BIAS)

            def edge_phase(_iv=None):
                psw = None
                for g in range(n_groups):
                    tlo, thi = g * GRP, min(T, g * GRP + GRP)
                    ng = thi - tlo
                    zs_g = zsp.tile([P, GRP * ZS], F16, tag="zs")
                    nc.sync.dma_start(out=zs_g[:, :ng * ZS],
                                      in_=zs[:, tlo * ZS:thi * ZS])
                    hs_g = hsp.tile([P, GRP * HC], F16, tag="hs")
                    nc.sync.dma_start(out=hs_g[:, :ng * HC],
                                      in_=hsrc[:, tlo * HC:thi * HC])
                    s8_g = s8p.tile([P, GRP * P], F8, tag="s8")
                    nc.sync.dma_start(out=s8_g[:, :ng * P],
                                      in_=s8[:, tlo * P:thi * P])

                    zs_r = zs_g[:].rearrange("p (t z) -> p t z", t=GRP)
                    zp_g = zpp.tile([P, GRP * H], F16, tag="zp")
                    zp_r = zp_g[:].rearrange("p (t h) -> p t h", t=GRP)
                    nc.vector.tensor_tensor(
                        out=zp_r[:, :ng, :], in0=zs_r[:, :ng, 0:8],
                        in1=zs_r[:, :ng, 8:16], op=OP.add)
                    nc.scalar.activation(zp_g[:, :ng * H], zp_g[:, :ng * H],
                                         AF.Prelu, alpha=NEG_SLOPE)

                    # ONE ACT op computes exp(z-4) broadcast-expanded over
                    # the 16 dims of each head in (c,h) order, including the
                    # compact denominator block at c=16 (cols 128:136).
                    msg_g = msgp.tile([P, GRP * SLOT], F16, tag="msg")
                    zb = zp_r[:, :ng, :]
                    zp_b = bass.AP(zb.tensor, zb.offset,
                                   [zb.ap[0], zb.ap[1], [0, 17], zb.ap[2]])
                    msg_r = msg_g[:].rearrange("p (t f) -> p t f", t=GRP)
                    mr = msg_r[:, :ng, :]
                    msg_chr = bass.AP(mr.tensor, mr.offset,
                                      [mr.ap[0], mr.ap[1], [8, 17], [1, 8]])
                    nc.scalar.activation(msg_chr, zp_b, AF.Exp,
                                         bias=ebias_sb[:])
                    hs_r = hs_g[:].rearrange("p (t c) -> p t c", t=GRP)
                    nc.vector.tensor_tensor(
                        out=msg_r[:, :ng, 0:HC], in0=hs_r[:, :ng, :],
                        in1=msg_r[:, :ng, 0:HC], op=OP.mult)

                    for j, t in enumerate(range(tlo, thi)):
                        w = tile_win[t]
                        if t == first_of_win[w]:
                            psw = psW.tile([P, SLOT], F32, tag="psw")
                        nc.tensor.matmul(
                            psw[:], s8_g[:, j * P:(j + 1) * P],
                            msg_g[:, j * SLOT:(j + 1) * SLOT],
                            start=(t == first_of_win[w]),
                            stop=(t == last_of_win[w]))
                        if t == last_of_win[w]:
                            den = epip.tile([P, H], F32, tag="den")
                            nc.scalar.activation(den[:], psw[:, HC:HC + H],
                                                 AF.Copy, bias=EPS)
                            rec = epip.tile([P, H], F16, tag="rec")
                            with nc.allow_low_precision(
                                    reason="softmax denominators are O(1)"):
                                nc.vector.reciprocal(rec[:], den[:])
                            o1p = epip.tile([P, HC], F16, tag="o1p")
                            nc.scalar.activation(o1p[:], psw[:, 0:HC],
                                                 AF.Copy)
                            r_ap = rec[:]
                            r_b = bass.AP(r_ap.tensor, r_ap.offset,
                                          [r_ap.ap[0], [0, 16], [1, H]])
                            o1 = epip.tile([P, HC], F16, tag="o1")
                            o1_r = o1[:].rearrange("p (c h) -> p c h", c=16)
                            o1p_r = o1p[:].rearrange("p (c h) -> p c h", c=16)
                            nc.vector.tensor_tensor(
                                out=o1_r, in0=o1p_r, in1=r_b, op=OP.mult)
                            nc.scalar.dma_start(
                                out=out[w * P:(w + 1) * P, :], in_=o1[:])

            if bench_loop > 1:
                with tc.For_i(0, bench_loop, 1) as _iv:
                    edge_phase(_iv)
            else:
                edge_phase()
    _finalize_kernel(nc)
    return nc


def _build_edge2(T, PC, wpc, bias_out, bench_loop=1):
    """Layer-2 edge aggregation, 1 head x 64ch. Messages are the streamed
    h2[src] (with a host-appended ones column for the denominator) scaled
    by the broadcast exp(z); one mixed fp8xfp16 matmul per tile against the
    streamed one-hot selection matrix."""
    C, CW, ZS = 64, 65, 2
    nc = bass.Bass()
    hsrc = nc.dram_tensor("hsrc", [P, T * CW], F16, kind="ExternalInput")
    s8 = nc.dram_tensor("s8", [P, T * P], F8, kind="ExternalInput")
    zs = nc.dram_tensor("zs", [P, T * ZS], F16, kind="ExternalInput")
    if bias_out:
        brep = nc.dram_tensor("brep", [P, C], F32, kind="ExternalInput")
    out = nc.dram_tensor("out", [wpc * P, C], F32, kind="ExternalOutput")

    n_groups = (T + GRP - 1) // GRP
    tile_win, first_of_win, last_of_win = _tile_windows(T, PC, wpc)

    with tile.TileContext(nc) as tc:
        with (
            tc.tile_pool(name="const", bufs=1) as constp,
            tc.tile_pool(name="zs", bufs=4) as zsp,
            tc.tile_pool(name="hs", bufs=4) as hsp,
            tc.tile_pool(name="s8", bufs=4) as s8p,
            tc.tile_pool(name="zp", bufs=3) as zpp,
            tc.tile_pool(name="msg", bufs=4) as msgp,
            tc.tile_pool(name="epi", bufs=3) as epip,
            tc.tile_pool(name="psW", bufs=3, space="PSUM") as psW,
        ):
            ebias_sb = constp.tile([P, 1], F32)
            nc.vector.memset(ebias_sb[:], EXP_BIAS)
            if bias_out:
                brep_sb = constp.tile([P, C], F32)
                nc.sync.dma_start(out=brep_sb[:], in_=brep[:])

            def edge_phase(_iv=None):
                psw = None
                for g in range(n_groups):
                    tlo, thi = g * GRP, min(T, g * GRP + GRP)
                    ng = thi - tlo
                    zs_g = zsp.tile([P, GRP * ZS], F16, tag="zs")
                    nc.sync.dma_start(out=zs_g[:, :ng * ZS],
                                      in_=zs[:, tlo * ZS:thi * ZS])
                    hs_g = hsp.tile([P, GRP * CW], F16, tag="hs")
                    nc.sync.dma_start(out=hs_g[:, :ng * CW],
                                      in_=hsrc[:, tlo * CW:thi * CW])
                    s8_g = s8p.tile([P, GRP * P], F8, tag="s8")
                    nc.sync.dma_start(out=s8_g[:, :ng * P],
                                      in_=s8[:, tlo * P:thi * P])

                    zs_r = zs_g[:].rearrange("p (t z) -> p t z", t=GRP)
                    zp_g = zpp.tile([P, GRP], F16, tag="zp")
                    zp_r = zp_g[:].rearrange("p (t z) -> p t z", z=1)
                    nc.vector.tensor_tensor(
                        out=zp_r[:, :ng], in0=zs_r[:, :ng, 0:1],
                        in1=zs_r[:, :ng, 1:2], op=OP.add)
                    nc.scalar.activation(zp_g[:, :ng], zp_g[:, :ng],
                                         AF.Prelu, alpha=NEG_SLOPE)
                    nc.scalar.activation(zp_g[:, :ng], zp_g[:, :ng], AF.Exp,
                                         bias=ebias_sb[:])

                    # msg = h2src * exp(z) broadcast over the 65 columns
                    msg_g = msgp.tile([P, GRP * CW], F16, tag="msg")
                    msg_r = msg_g[:].rearrange("p (t c) -> p t c", t=GRP)
                    hs_r = hs_g[:].rearrange("p (t c) -> p t c", t=GRP)
                    zb = zp_r[:, :ng]
                    zp_b = bass.AP(zb.tensor, zb.offset,
                                   [zb.ap[0], zb.ap[1], [0, CW]])
                    nc.vector.tensor_tensor(
                        out=msg_r[:, :ng, :], in0=hs_r[:, :ng, :],
                        in1=zp_b, op=OP.mult)

                    for j, t in enumerate(range(tlo, thi)):
                        w = tile_win[t]
                        if t == first_of_win[w]:
                            psw = psW.tile([P, CW], F32, tag="psw")
                        nc.tensor.matmul(
                            psw[:], s8_g[:, j * P:(j + 1) * P],
                            msg_g[:, j * CW:(j + 1) * CW],
                            start=(t == first_of_win[w]),
                            stop=(t == last_of_win[w]))
                        if t == last_of_win[w]:
                            den = epip.tile([P, 1], F32, tag="den")
                            nc.scalar.activation(den[:], psw[:, C:C + 1],
                                                 AF.Copy, bias=EPS)
                            rec = epip.tile([P, 1], F32, tag="rec")
                            nc.vector.reciprocal(rec[:], den[:])
                            r_ap = rec[:]
                            r_b = bass.AP(r_ap.tensor, r_ap.offset,
                                          [r_ap.ap[0], [0, C]])
                            o2 = epip.tile([P, C], F32, tag="o2")
                            nc.vector.tensor_tensor(
                                out=o2[:], in0=psw[:, 0:C], in1=r_b,
                                op=OP.mult)
                            if bias_out:
                                nc.vector.tensor_tensor(
                                    out=o2[:], in0=o2[:], in1=brep_sb[:],
                                    op=OP.add)
                            nc.scalar.dma_start(
                                out=out[w * P:(w + 1) * P, :], in_=o2[:])

            if bench_loop > 1:
                with tc.For_i(0, bench_loop, 1) as _iv:
                    edge_phase(_iv)
            else:
                edge_phase()
    _finalize_kernel(nc)
    return nc


# ------------------------------------------------------------------ runner

def _fold_att(W, a):
    heads, hid = a.shape
    return np.einsum("ihc,hc->ih", W.reshape(W.shape[0], heads, hid), a)


class _GatRunner:
    def __init__(self, n_cores=N_CORES):
        self.C = n_cores
        self._graph = None
        self._graph_key = None
        self._kernels = {}
        self.last_maps = {}

    def graph(self, edge_index, n_nodes):
        key = hash(np.asarray(edge_index).tobytes())
        if key != self._graph_key:
            self._graph = _Graph(edge_index, n_nodes, self.C)
            self._graph_key = key
            self._kernels.clear()
        return self._graph

    def kernel(self, name, bench_loop=1, **kw):
        key = (name, bench_loop, tuple(sorted(kw.items())))
        if key not in self._kernels:
            g = self._graph
            if name.startswith("P"):
                self._kernels[key] = _build_node(
                    g.shard_nodes, bench_loop=bench_loop, **kw)
            elif name == "E1":
                self._kernels[key] = _build_edge1(
                    g.T, g.PC, g.wpc, bench_loop=bench_loop)
            else:
                self._kernels[key] = _build_edge2(
                    g.T, g.PC, g.wpc, bench_loop=bench_loop, **kw)
        return self._kernels[key]

    def _run(self, name, nc, maps):
        self.last_maps[name] = maps
        res = run_bass_kernel_spmd(nc, maps, core_ids=list(range(self.C)))
        return res.results

    def run(self, x, edge_index, W1, a_src1, a_dst1, b1, W2, a_src2, a_dst2,
            b2):
        C = self.C
        N, IN_C = x.shape
        HEADS, HID = a_src1.shape
        HC = HEADS * HID
        OUT_C = W2.shape[1]
        g = self.graph(edge_index, N)
        SH = g.shard_nodes
        # (c,h)-interleaved channel order for the layer-1 hidden features:
        # col c*H+h of h1 holds math channel h*HID+c. Folded into W1's
        # columns (P0) and W2's rows (P2) on the host - pure permutation.
        perm = np.array([(j % HEADS) * HID + j // HEADS
                         for j in range(HC)], dtype=np.int64)

        # ---- P0: per-node h1 / logits --------------------------------
        xT_pad = np.zeros((IN_C, g.n_pad), dtype=np.float16)
        xT_pad[:, :N] = np.asarray(x, np.float32).T
        w1 = np.asarray(W1, np.float32)
        wal1 = np.concatenate(
            [_fold_att(w1, np.asarray(a_src1, np.float32)),
             _fold_att(w1, np.asarray(a_dst1, np.float32))], axis=1)
        mapsP0 = [{"xT": np.ascontiguousarray(xT_pad[:, k * SH:(k + 1) * SH]),
                   "w": np.ascontiguousarray(w1[:, perm]).astype(np.float16),
                   "wal": wal1.astype(np.float16)} for k in range(C)]
        ncP0 = self.kernel("P0", c_in=IN_C, m_h=HC, m_al=2 * HEADS,
                           elu=False, bias_in=False)
        resP0 = self._run("P0", ncP0, mapsP0)
        h1 = np.ascontiguousarray(
            np.concatenate([r["hT"] for r in resP0], axis=1).T)  # [Np,HC] f16
        al1 = np.concatenate([r["alT"] for r in resP0], axis=1)  # [16,Np] f16
        als1 = np.ascontiguousarray(al1[:HEADS].T)
        ald1 = np.ascontiguousarray(al1[HEADS:].T)

        # ---- E1: layer-1 edge aggregation ----------------------------
        mapsE1 = [{"hsrc": g.stream_feat(h1, k),
                   "s8": g.stream_sel(k),
                   "zs": g.stream_zs(als1, ald1, k)} for k in range(C)]
        ncE1 = self.kernel("E1")
        resE1 = self._run("E1", ncE1, mapsE1)
        out1 = np.concatenate([r["out"] for r in resE1], axis=0)  # [Np, HC]

        # ---- P2: ELU + per-node h2 / logits --------------------------
        o1T = np.ascontiguousarray(out1.T)  # [HC, Np] f16, (c,h) rows
        w2 = np.asarray(W2, np.float32)
        wal2 = np.concatenate(
            [_fold_att(w2, np.asarray(a_src2, np.float32)),
             _fold_att(w2, np.asarray(a_dst2, np.float32))], axis=1)
        b1nz = bool(np.any(np.asarray(b1)))
        w2all = np.concatenate([w2[perm], wal2[perm]], axis=1)  # [HC, 66]
        mapsP2 = []
        for k in range(C):
            m = {"xT": np.ascontiguousarray(o1T[:, k * SH:(k + 1) * SH]),
                 "w": w2all.astype(np.float16)}
            if b1nz:
                m["bvec"] = np.asarray(b1, np.float32)[perm].reshape(HC, 1)
            mapsP2.append(m)
        ncP2 = self.kernel("P2", c_in=HC, m_h=OUT_C, m_al=2, elu=True,
                           bias_in=b1nz)
        resP2 = self._run("P2", ncP2, mapsP2)
        h2al = np.concatenate([r["hT"] for r in resP2], axis=1)  # [66, Np]
        h2 = np.ascontiguousarray(h2al[:OUT_C].T)  # [Np, 64] f16
        als2, ald2 = h2al[OUT_C], h2al[OUT_C + 1]

        # ---- E2: layer-2 edge aggregation ----------------------------
        b2nz = bool(np.any(np.asarray(b2)))
        mapsE2 = []
        for k in range(C):
            m = {"hsrc": g.stream_feat(h2, k, ones_col=True),
                 "s8": g.stream_sel(k),
                 "zs": g.stream_zs2(als2, ald2, k)}
            if b2nz:
                m["brep"] = np.tile(np.asarray(b2, np.float32), (P, 1))
            mapsE2.append(m)
        ncE2 = self.kernel("E2", bias_out=b2nz)
        resE2 = self._run("E2", ncE2, mapsE2)
        out2 = np.concatenate([r["out"] for r in resE2], axis=0)
        return out2[:N]


_RUNNER = _GatRunner()


def kernel(x, edge_index, W1, a_src1, a_dst1, b1, W2, a_src2, a_dst2, b2):
    """Full-input / full-output entry point. Returns [N, OUT_C] float32."""
    args = [np.asarray(v) for v in
            (x, edge_index, W1, a_src1, a_dst1, b1, W2, a_src2, a_dst2, b2)]
    return _RUNNER.run(*args).astype(np.float32)


# revision 19
# speedup vs baseline: 1.6803x; 1.1095x over previous
"""Trainium (trn2) Bass kernel for a 2-layer GAT over N=100k nodes / E=1.7M edges.

Strategy (node-transform / edge-aggregate split, streamed fp8 selection)
------------------------------------------------------------------------
Edges are sorted by destination on the host (index-only preprocessing); the
destination axis is sharded across the 8 NeuronCores in contiguous 128-node
windows (98 windows per core).  Each GAT layer runs as TWO SPMD kernels with
host-side index gathers (pure permutations / casts - no host FLOPs) between
them:

* node kernel (P0/P2): h = x @ W and the folded attention logits
  al_s = x @ (W a_s), al_d = x @ (W a_d), computed ONCE PER NODE (dense
  matmuls, ~50 us/core).  P2 fuses the ELU of the layer-1 output into its
  input stream and merges the al columns into the main matmul (64+2<=128).
* host: gathers per-edge streams h[src], al_s[src], al_d[dst] into the
  dst-sorted slot order (numpy fancy indexing = permutation only), and
  prebuilds a graph-constant fp8 one-hot selection stream
  S[e, t*128+n] = (rel_dst==n) shared by both layers.
* edge kernel (E1/E2): per 32-tile group, z = al_s+al_d (DVE 2x),
  leaky_relu on ACT, then ONE ACT Exp op writes exp(z-4) broadcast over
  each head's 16 dims in (c,h)-interleaved channel order plus the compact
  denominator block (cols 128:136); the constant -4 bias keeps fp16 exp
  in range and cancels in the softmax.  One in-place DVE multiply (2x
  mode, all unit-stride) forms the messages, and one mixed fp8xfp16
  matmul per 128-edge tile (fp8 stationary -> fast weight load)
  accumulates [msg | exp] into the window's PSUM slot.  E2 (1 head)
  broadcasts exp(z) straight into the multiply and carries the
  denominator on a host-appended ones column.  Epilogues split across
  ACT (PSUM reads, +eps via Copy bias) and DVE (reciprocal, scale).
  Input streams ride the SP hardware DMA queue; output DMAs ride the
  ACT queue so window outputs never head-of-line-block the streams.

The (c,h) channel interleave is free: W1's columns and W2's rows are
permuted on the host.  Edge kernels run at the HBM stream floor (~317
GB/s/core measured): h[src] fp16 + S fp8 + logits = ~48 KB per 128 edges.

Environment workarounds: this container's walrus build allows only ONE
semaphore wait per instruction (split onto nop carriers post-scheduling), and
the GPSIMD ucode libraries are absent (so no dma_gather/indirect-DMA fast
paths - hence the host-gather design).
"""
import numpy as np

import concourse.bass as bass
import concourse.mybir as mybir
import concourse.tile as tile
from concourse.bass_utils import run_bass_kernel_spmd

P = 128
F16 = mybir.dt.float16
F32 = mybir.dt.float32
F8 = mybir.dt.float8e4
AF = mybir.ActivationFunctionType
OP = mybir.AluOpType
NEG_SLOPE = 0.2
EXP_BIAS = -4.0     # exp(z + EXP_BIAS): constant shift cancels in softmax
GRP = 32            # tiles per stream group
PAD_REL = 255.0     # rel value for pad slots -> is_equal never matches
N_CORES = 8
EPS = 1e-30
CH = 512            # node-kernel chunk (one PSUM bank of fp32)

# ------------------------------------------------------------------ patches

_wsplit_counter = [0]


def _split_excess_waits(nc, max_waits=1):
    """This walrus build rejects >1 sem-wait per instruction ("Too many sync
    wait commands"). Move overflow waits onto same-engine nop carriers."""
    n_split = 0
    for f in nc.m.functions:
        for blk in f.blocks:
            changed = False
            out = []
            for inst in blk.instructions:
                si = inst.sync_info
                if si is not None and len(si.on_wait) > max_waits:
                    waits = list(si.on_wait)
                    keep = waits[len(waits) - max_waits:]
                    overflow = waits[: len(waits) - max_waits]
                    for i in range(0, len(overflow), max_waits):
                        _wsplit_counter[0] += 1
                        nop = mybir.InstNoOp(
                            name=f"I-wsplit-{_wsplit_counter[0]}", ins=[], outs=[])
                        nop.engine = inst.engine
                        nop.sync_info = mybir.SyncInfo(
                            on_wait=overflow[i: i + max_waits], on_update=[])
                        out.append(nop)
                    inst.sync_info = mybir.SyncInfo(
                        on_wait=keep, on_update=list(si.on_update))
                    changed = True
                    n_split += 1
                out.append(inst)
            if changed:
                blk.instructions = out
    return n_split


def _finalize_kernel(nc):
    import bass_rust as _bass_rust
    from concourse.library_config import all_libraries, standard
    from concourse.library_overlay import lower_extended_insts

    inst_type_to_lib_mask = {}
    for lib in all_libraries:
        for inst_type in lib.instructions:
            inst_type_to_lib_mask[inst_type] = inst_type_to_lib_mask.get(
                inst_type, 0) | (1 << lib.index)
    _bass_rust.insert_library_loads(
        nc, inst_type_to_lib_mask, len(all_libraries), standard.index)
    lower_extended_insts(nc)
    _split_excess_waits(nc)


# ------------------------------------------------------------------ host prep

class _Graph:
    """Host-side index preprocessing: sort by dst, shard dst windows across
    cores, pad per-window tile counts to a global schedule so all cores run
    one identical SPMD program."""

    def __init__(self, edge_index, n_nodes, n_cores):
        self.N = n_nodes
        self.C = n_cores
        src = np.asarray(edge_index[0], dtype=np.int64)
        dst = np.asarray(edge_index[1], dtype=np.int64)
        perm = np.argsort(dst, kind="stable")
        self.src_s = src[perm].astype(np.int32)
        self.dst_s = dst[perm].astype(np.int32)

        n_win_total = (n_nodes + P - 1) // P
        self.wpc = (n_win_total + n_cores - 1) // n_cores
        self.n_win = self.wpc * n_cores
        self.shard_nodes = self.wpc * P
        self.n_pad = self.n_win * P

        bounds = np.searchsorted(self.dst_s, np.arange(0, self.n_win + 1) * P)
        counts = np.zeros((n_cores, self.wpc), dtype=np.int64)
        for k in range(n_cores):
            for i in range(self.wpc):
                w = k * self.wpc + i
                if w < n_win_total:
                    counts[k, i] = bounds[w + 1] - bounds[w]
        self.PC = np.maximum(np.ceil(counts / P).astype(np.int64).max(axis=0), 1)
        self.T = int(self.PC.sum())

        self.slot_src = np.zeros((n_cores, self.T * P), dtype=np.int32)
        self.slot_dst = np.zeros((n_cores, self.T * P), dtype=np.int32)
        self.slot_rel = np.full((n_cores, self.T * P), int(PAD_REL), dtype=np.int32)
        for k in range(n_cores):
            t0 = 0
            for i in range(self.wpc):
                w = k * self.wpc + i
                cnt = int(counts[k, i])
                if cnt > 0:
                    e0 = bounds[w]
                    sl = t0 * P
                    self.slot_src[k, sl:sl + cnt] = self.src_s[e0:e0 + cnt]
                    self.slot_dst[k, sl:sl + cnt] = self.dst_s[e0:e0 + cnt]
                    self.slot_rel[k, sl:sl + cnt] = self.dst_s[e0:e0 + cnt] - w * P
                t0 += int(self.PC[i])
        self.src2d = self.slot_src.reshape(n_cores, self.T, P)
        self.dst2d = self.slot_dst.reshape(n_cores, self.T, P)
        self.rel2d = self.slot_rel.reshape(n_cores, self.T, P)

    def stream_feat(self, table, core, ones_col=False):
        """[128, T*C] (or T*(C+1) with a trailing ones column per tile):
        col t*C+c of partition e = table[src[slot t,e], c]."""
        T, C = self.T, table.shape[1]
        W = C + 1 if ones_col else C
        out = np.empty((T, P, W), dtype=np.float16)
        out[:, :, :C] = table[self.src2d[core]]
        if ones_col:
            out[:, :, C] = 1.0
        return np.ascontiguousarray(out.transpose(1, 0, 2)).reshape(P, T * W)

    def stream_zs(self, als, ald, core):
        """[128, T*16] f16: per tile [al_s[src] (8) | al_d[dst] (8)]."""
        T = self.T
        z = np.empty((T, P, 16), dtype=np.float16)
        z[:, :, 0:8] = als[self.src2d[core]]
        z[:, :, 8:16] = ald[self.dst2d[core]]
        return np.ascontiguousarray(z.transpose(1, 0, 2)).reshape(P, T * 16)

    def stream_zs2(self, als, ald, core):
        """[128, T*2] f16: per tile [al_s[src], al_d[dst]]."""
        T = self.T
        z = np.empty((T, P, 2), dtype=np.float16)
        z[:, :, 0] = als[self.src2d[core]]
        z[:, :, 1] = ald[self.dst2d[core]]
        return np.ascontiguousarray(z.transpose(1, 0, 2)).reshape(P, T * 2)

    def stream_sel(self, core):
        """[128, T*128] fp8e4m3 one-hot: col t*128+n of partition e is
        1.0 iff rel[t,e] == n. Graph-only; shared by both layers."""
        if not hasattr(self, "_sel"):
            self._sel = {}
        if core not in self._sel:
            import ml_dtypes
            one = np.float32(1.0).astype(ml_dtypes.float8_e4m3).view(np.uint8)
            T = self.T
            arr = np.zeros((T, P, P), dtype=np.uint8)
            rel = self.rel2d[core]
            t_i, e_i = np.nonzero(rel < P)
            arr[t_i, e_i, rel[t_i, e_i]] = one
            self._sel[core] = np.ascontiguousarray(
                arr.transpose(1, 0, 2)).reshape(P, T * P).view(
                    ml_dtypes.float8_e4m3)
        return self._sel[core]


# ------------------------------------------------------------------ builders

def _build_node(SH, c_in, m_h, m_al, elu, bias_in, bench_loop=1):
    """Per-node transform: hT = (elu?(xT+b)) @ w, alT = same @ wal.
    When m_h+m_al <= 128 the two matmuls merge into one (w carries the
    al columns and alT is folded into hT's extra rows)."""
    merged = (m_h + m_al) <= P
    M = m_h + m_al if merged else m_h
    nc = bass.Bass()
    xT = nc.dram_tensor("xT", [c_in, SH], F16, kind="ExternalInput")
    w = nc.dram_tensor("w", [c_in, M], F16, kind="ExternalInput")
    if not merged:
        wal = nc.dram_tensor("wal", [c_in, m_al], F16, kind="ExternalInput")
    if bias_in:
        bvec = nc.dram_tensor("bvec", [c_in, 1], F32, kind="ExternalInput")
    hT = nc.dram_tensor("hT", [M, SH], F16, kind="ExternalOutput")
    if not merged:
        alT = nc.dram_tensor("alT", [m_al, SH], F16, kind="ExternalOutput")

    with tile.TileContext(nc) as tc:
        with (
            tc.tile_pool(name="const", bufs=1) as constp,
            tc.tile_pool(name="xs", bufs=4) as xsp,
            tc.tile_pool(name="work", bufs=4) as workp,
            tc.tile_pool(name="out", bufs=4) as outp,
            tc.tile_pool(name="psH", bufs=3, space="PSUM") as psH,
            tc.tile_pool(name="psA", bufs=3, space="PSUM") as psA,
        ):
            w_sb = constp.tile([c_in, M], F16)
            nc.sync.dma_start(out=w_sb[:], in_=w[:])
            if not merged:
                wal_sb = constp.tile([c_in, m_al], F16)
                nc.sync.dma_start(out=wal_sb[:], in_=wal[:])
            if bias_in:
                b_sb = constp.tile([c_in, 1], F32)
                nc.sync.dma_start(out=b_sb[:], in_=bvec[:])

            def body(_iv=None):
                for c0 in range(0, SH, CH):
                    nb = min(CH, SH - c0)
                    xc = xsp.tile([c_in, CH], F16, tag="xc")
                    nc.sync.dma_start(out=xc[:, :nb], in_=xT[:, c0:c0 + nb])
                    rhs = xc
                    if elu:
                        if bias_in:
                            nc.vector.tensor_scalar(
                                xc[:, :nb], xc[:, :nb], b_sb[:, 0:1], None,
                                OP.add)
                        mn = workp.tile([c_in, CH], F16, tag="mn")
                        nc.vector.tensor_scalar(
                            mn[:, :nb], xc[:, :nb], 0.0, None, OP.min)
                        nc.scalar.activation(mn[:, :nb], mn[:, :nb], AF.Exp)
                        mx = workp.tile([c_in, CH], F16, tag="mx")
                        nc.vector.tensor_scalar(
                            mx[:, :nb], xc[:, :nb], 0.0, -1.0, OP.max, OP.add)
                        xe = workp.tile([c_in, CH], F16, tag="xe")
                        nc.vector.tensor_tensor(
                            out=xe[:, :nb], in0=mx[:, :nb], in1=mn[:, :nb],
                            op=OP.add)
                        rhs = xe
                    ph = psH.tile([M, CH], F32, tag="ph")
                    nc.tensor.matmul(ph[:, :nb], w_sb[:], rhs[:, :nb],
                                     start=True, stop=True)
                    h_sb = outp.tile([M, CH], F16, tag="h")
                    nc.scalar.activation(h_sb[:, :nb], ph[:, :nb], AF.Copy)
                    nc.scalar.dma_start(out=hT[:, c0:c0 + nb],
                                        in_=h_sb[:, :nb])
                    if not merged:
                        pa = psA.tile([m_al, CH], F32, tag="pa")
                        nc.tensor.matmul(pa[:, :nb], wal_sb[:], rhs[:, :nb],
                                         start=True, stop=True)
                        a_sb = outp.tile([m_al, CH], F16, tag="a")
                        nc.vector.tensor_copy(a_sb[:, :nb], pa[:, :nb])
                        nc.scalar.dma_start(out=alT[:, c0:c0 + nb],
                                            in_=a_sb[:, :nb])

            if bench_loop > 1:
                with tc.For_i(0, bench_loop, 1) as _iv:
                    body(_iv)
            else:
                body()
    _finalize_kernel(nc)
    return nc


def _tile_windows(T, PC, wpc):
    tile_win = []
    for i in range(wpc):
        tile_win += [i] * int(PC[i])
    first_of_win, last_of_win = {}, {}
    for t, w in enumerate(tile_win):
        first_of_win.setdefault(w, t)
        last_of_win[w] = t
    return tile_win, first_of_win, last_of_win


def _build_edge1(T, PC, wpc, bench_loop=1):
    """Layer-1 edge aggregation, 8 heads x 16ch, (c,h)-interleaved channel
    order (channel c*8+h = head h, dim c). Streams h1[src], the fp8 one-hot
    selection matrix, and the logit pairs; one mixed fp8xfp16 matmul per
    128-edge tile accumulates [msg | exp] into the window's PSUM slot.
    Output is the PRE-ELU aggregated feature in (c,h) order."""
    HC, H, ZS, SLOT = 128, 8, 16, 136
    nc = bass.Bass()
    hsrc = nc.dram_tensor("hsrc", [P, T * HC], F16, kind="ExternalInput")
    s8 = nc.dram_tensor("s8", [P, T * P], F8, kind="ExternalInput")
    zs = nc.dram_tensor("zs", [P, T * ZS], F16, kind="ExternalInput")
    out = nc.dram_tensor("out", [wpc * P, HC], F16, kind="ExternalOutput")

    n_groups = (T + GRP - 1) // GRP
    tile_win, first_of_win, last_of_win = _tile_windows(T, PC, wpc)

    with tile.TileContext(nc) as tc:
        with (
            tc.tile_pool(name="const", bufs=1) as constp,
            tc.tile_pool(name="zs", bufs=4) as zsp,
            tc.tile_pool(name="hs", bufs=4) as hsp,
            tc.tile_pool(name="s8", bufs=4) as s8p,
            tc.tile_pool(name="zp", bufs=3) as zpp,
            tc.tile_pool(name="msg", bufs=4) as msgp,
            tc.tile_pool(name="epi", bufs=3) as epip,
            tc.tile_pool(name="psW", bufs=3, space="PSUM") as psW,
        ):
            ebias_sb = constp.tile([P, 1], F32)
            nc.vector.memset(ebias_sb[:], EXP_BIAS)

            def edge_phase(_iv=None):
                psw = None
                for g in range(n_groups):
                    tlo, thi = g * GRP, min(T, g * GRP + GRP)
                    ng = thi - tlo
                    zs_g = zsp.tile([P, GRP * ZS], F16, tag="zs")
                    nc.sync.dma_start(out=zs_g[:, :ng * ZS],
                                      in_=zs[:, tlo * ZS:thi * ZS])
                    hs_g = hsp.tile([P, GRP * HC], F16, tag="hs")
                    nc.sync.dma_start(out=hs_g[:, :ng * HC],
                                      in_=hsrc[:, tlo * HC:thi * HC])
                    s8_g = s8p.tile([P, GRP * P], F8, tag="s8")
                    nc.sync.dma_start(out=s8_g[:, :ng * P],
                                      in_=s8[:, tlo * P:thi * P])

                    zs_r = zs_g[:].rearrange("p (t z) -> p t z", t=GRP)
                    zp_g = zpp.tile([P, GRP * H], F16, tag="zp")
                    zp_r = zp_g[:].rearrange("p (t h) -> p t h", t=GRP)
                    nc.vector.tensor_tensor(
                        out=zp_r[:, :ng, :], in0=zs_r[:, :ng, 0:8],
                        in1=zs_r[:, :ng, 8:16], op=OP.add)
                    nc.scalar.activation(zp_g[:, :ng * H], zp_g[:, :ng * H],
                                         AF.Prelu, alpha=NEG_SLOPE)

                    # ONE ACT op computes exp(z-4) broadcast-expanded over
                    # the 16 dims of each head in (c,h) order, including the
                    # compact denominator block at c=16 (cols 128:136).
                    msg_g = msgp.tile([P, GRP * SLOT], F16, tag="msg")
                    zb = zp_r[:, :ng, :]
                    zp_b = bass.AP(zb.tensor, zb.offset,
                                   [zb.ap[0], zb.ap[1], [0, 17], zb.ap[2]])
                    msg_r = msg_g[:].rearrange("p (t f) -> p t f", t=GRP)
                    mr = msg_r[:, :ng, :]
                    msg_chr = bass.AP(mr.tensor, mr.offset,
                                      [mr.ap[0], mr.ap[1], [8, 17], [1, 8]])
                    nc.scalar.activation(msg_chr, zp_b, AF.Exp,
                                         bias=ebias_sb[:])
                    hs_r = hs_g[:].rearrange("p (t c) -> p t c", t=GRP)
                    nc.vector.tensor_tensor(
                        out=msg_r[:, :ng, 0:HC], in0=hs_r[:, :ng, :],
                        in1=msg_r[:, :ng, 0:HC], op=OP.mult)

                    for j, t in enumerate(range(tlo, thi)):
                        w = tile_win[t]
                        if t == first_of_win[w]:
                            psw = psW.tile([P, SLOT], F32, tag="psw")
                        nc.tensor.matmul(
                            psw[:], s8_g[:, j * P:(j + 1) * P],
                            msg_g[:, j * SLOT:(j + 1) * SLOT],
                            start=(t == first_of_win[w]),
                            stop=(t == last_of_win[w]))
                        if t == last_of_win[w]:
                            den = epip.tile([P, H], F32, tag="den")
                            nc.scalar.activation(den[:], psw[:, HC:HC + H],
                                                 AF.Copy, bias=EPS)
                            rec = epip.tile([P, H], F16, tag="rec")
                            with nc.allow_low_precision(
                                    reason="softmax denominators are O(1)"):
                                nc.vector.reciprocal(rec[:], den[:])
                            o1p = epip.tile([P, HC], F16, tag="o1p")
                            nc.scalar.activation(o1p[:], psw[:, 0:HC],
                                                 AF.Copy)
                            r_ap = rec[:]
                            r_b = bass.AP(r_ap.tensor, r_ap.offset,
                                          [r_ap.ap[0], [0, 16], [1, H]])
                            o1 = epip.tile([P, HC], F16, tag="o1")
                            o1_r = o1[:].rearrange("p (c h) -> p c h", c=16)
                            o1p_r = o1p[:].rearrange("p (c h) -> p c h", c=16)
                            nc.vector.tensor_tensor(
                                out=o1_r, in0=o1p_r, in1=r_b, op=OP.mult)
                            nc.scalar.dma_start(
                                out=out[w * P:(w + 1) * P, :], in_=o1[:])

            if bench_loop > 1:
                with tc.For_i(0, bench_loop, 1) as _iv:
                    edge_phase(_iv)
            else:
                edge_phase()
    _finalize_kernel(nc)
    return nc


def _build_edge2(T, PC, wpc, bias_out, bench_loop=1):
    """Layer-2 edge aggregation, 1 head x 64ch. Messages are the streamed
    h2[src] (with a host-appended ones column for the denominator) scaled
    by the broadcast exp(z); one mixed fp8xfp16 matmul per tile against the
    streamed one-hot selection matrix."""
    C, CW, ZS = 64, 65, 2
    nc = bass.Bass()
    hsrc = nc.dram_tensor("hsrc", [P, T * CW], F16, kind="ExternalInput")
    s8 = nc.dram_tensor("s8", [P, T * P], F8, kind="ExternalInput")
    zs = nc.dram_tensor("zs", [P, T * ZS], F16, kind="ExternalInput")
    if bias_out:
        brep = nc.dram_tensor("brep", [P, C], F32, kind="ExternalInput")
    out = nc.dram_tensor("out", [wpc * P, C], F32, kind="ExternalOutput")

    n_groups = (T + GRP - 1) // GRP
    tile_win, first_of_win, last_of_win = _tile_windows(T, PC, wpc)

    with tile.TileContext(nc) as tc:
        with (
            tc.tile_pool(name="const", bufs=1) as constp,
            tc.tile_pool(name="zs", bufs=4) as zsp,
            tc.tile_pool(name="hs", bufs=4) as hsp,
            tc.tile_pool(name="s8", bufs=4) as s8p,
            tc.tile_pool(name="zp", bufs=3) as zpp,
            tc.tile_pool(name="msg", bufs=4) as msgp,
            tc.tile_pool(name="epi", bufs=3) as epip,
            tc.tile_pool(name="psW", bufs=3, space="PSUM") as psW,
        ):
            ebias_sb = constp.tile([P, 1], F32)
            nc.vector.memset(ebias_sb[:], EXP_BIAS)
            if bias_out:
                brep_sb = constp.tile([P, C], F32)
                nc.sync.dma_start(out=brep_sb[:], in_=brep[:])

            def edge_phase(_iv=None):
                psw = None
                for g in range(n_groups):
                    tlo, thi = g * GRP, min(T, g * GRP + GRP)
                    ng = thi - tlo
                    zs_g = zsp.tile([P, GRP * ZS], F16, tag="zs")
                    nc.sync.dma_start(out=zs_g[:, :ng * ZS],
                                      in_=zs[:, tlo * ZS:thi * ZS])
                    hs_g = hsp.tile([P, GRP * CW], F16, tag="hs")
                    nc.sync.dma_start(out=hs_g[:, :ng * CW],
                                      in_=hsrc[:, tlo * CW:thi * CW])
                    s8_g = s8p.tile([P, GRP * P], F8, tag="s8")
                    nc.sync.dma_start(out=s8_g[:, :ng * P],
                                      in_=s8[:, tlo * P:thi * P])

                    zs_r = zs_g[:].rearrange("p (t z) -> p t z", t=GRP)
                    zp_g = zpp.tile([P, GRP], F16, tag="zp")
                    zp_r = zp_g[:].rearrange("p (t z) -> p t z", z=1)
                    nc.vector.tensor_tensor(
                        out=zp_r[:, :ng], in0=zs_r[:, :ng, 0:1],
                        in1=zs_r[:, :ng, 1:2], op=OP.add)
                    nc.scalar.activation(zp_g[:, :ng], zp_g[:, :ng],
                                         AF.Prelu, alpha=NEG_SLOPE)
                    nc.scalar.activation(zp_g[:, :ng], zp_g[:, :ng], AF.Exp,
                                         bias=ebias_sb[:])

                    # msg = h2src * exp(z) broadcast over the 65 columns
                    msg_g = msgp.tile([P, GRP * CW], F16, tag="msg")
                    msg_r = msg_g[:].rearrange("p (t c) -> p t c", t=GRP)
                    hs_r = hs_g[:].rearrange("p (t c) -> p t c", t=GRP)
                    zb = zp_r[:, :ng]
                    zp_b = bass.AP(zb.tensor, zb.offset,
                                   [zb.ap[0], zb.ap[1], [0, CW]])
                    nc.vector.tensor_tensor(
                        out=msg_r[:, :ng, :], in0=hs_r[:, :ng, :],
                        in1=zp_b, op=OP.mult)

                    for j, t in enumerate(range(tlo, thi)):
                        w = tile_win[t]
                        if t == first_of_win[w]:
                            psw = psW.tile([P, CW], F32, tag="psw")
                        nc.tensor.matmul(
                            psw[:], s8_g[:, j * P:(j + 1) * P],
                            msg_g[:, j * CW:(j + 1) * CW],
                            start=(t == first_of_win[w]),
                            stop=(t == last_of_win[w]))
                        if t == last_of_win[w]:
                            den = epip.tile([P, 1], F32, tag="den")
                            nc.scalar.activation(den[:], psw[:, C:C + 1],
                                                 AF.Copy, bias=EPS)
                            rec = epip.tile([P, 1], F32, tag="rec")
                            nc.vector.reciprocal(rec[:], den[:])
                            r_ap = rec[:]
                            r_b = bass.AP(r_ap.tensor, r_ap.offset,
                                          [r_ap.ap[0], [0, C]])
                            o2 = epip.tile([P, C], F32, tag="o2")
                            nc.vector.tensor_tensor(
                                out=o2[:], in0=psw[:, 0:C], in1=r_b,
                                op=OP.mult)
                            if bias_out:
                                nc.vector.tensor_tensor(
                                    out=o2[:], in0=o2[:], in1=brep_sb[:],
                                    op=OP.add)
                            nc.scalar.dma_start(
                                out=out[w * P:(w + 1) * P, :], in_=o2[:])

            if bench_loop > 1:
                with tc.For_i(0, bench_loop, 1) as _iv:
                    edge_phase(_iv)
            else:
                edge_phase()
    _finalize_kernel(nc)
    return nc


# ------------------------------------------------------------------ runner

def _fold_att(W, a):
    heads, hid = a.shape
    return np.einsum("ihc,hc->ih", W.reshape(W.shape[0], heads, hid), a)


class _GatRunner:
    def __init__(self, n_cores=N_CORES):
        self.C = n_cores
        self._graph = None
        self._graph_key = None
        self._kernels = {}
        self.last_maps = {}

    def graph(self, edge_index, n_nodes):
        key = hash(np.asarray(edge_index).tobytes())
        if key != self._graph_key:
            self._graph = _Graph(edge_index, n_nodes, self.C)
            self._graph_key = key
            self._kernels.clear()
        return self._graph

    def kernel(self, name, bench_loop=1, **kw):
        key = (name, bench_loop, tuple(sorted(kw.items())))
        if key not in self._kernels:
            g = self._graph
            if name.startswith("P"):
                self._kernels[key] = _build_node(
                    g.shard_nodes, bench_loop=bench_loop, **kw)
            elif name == "E1":
                self._kernels[key] = _build_edge1(
                    g.T, g.PC, g.wpc, bench_loop=bench_loop)
            else:
                self._kernels[key] = _build_edge2(
                    g.T, g.PC, g.wpc, bench_loop=bench_loop, **kw)
        return self._kernels[key]

    def _run(self, name, nc, maps):
        self.last_maps[name] = maps
        res = run_bass_kernel_spmd(nc, maps, core_ids=list(range(self.C)))
        return res.results

    def run(self, x, edge_index, W1, a_src1, a_dst1, b1, W2, a_src2, a_dst2,
            b2):
        C = self.C
        N, IN_C = x.shape
        HEADS, HID = a_src1.shape
        HC = HEADS * HID
        OUT_C = W2.shape[1]
        g = self.graph(edge_index, N)
        SH = g.shard_nodes
        # (c,h)-interleaved channel order for the layer-1 hidden features:
        # col c*H+h of h1 holds math channel h*HID+c. Folded into W1's
        # columns (P0) and W2's rows (P2) on the host - pure permutation.
        perm = np.array([(j % HEADS) * HID + j // HEADS
                         for j in range(HC)], dtype=np.int64)

        # ---- P0: per-node h1 / logits --------------------------------
        xT_pad = np.zeros((IN_C, g.n_pad), dtype=np.float16)
        xT_pad[:, :N] = np.asarray(x, np.float32).T
        w1 = np.asarray(W1, np.float32)
        wal1 = np.concatenate(
            [_fold_att(w1, np.asarray(a_src1, np.float32)),
             _fold_att(w1, np.asarray(a_dst1, np.float32))], axis=1)
        mapsP0 = [{"xT": np.ascontiguousarray(xT_pad[:, k * SH:(k + 1) * SH]),
                   "w": np.ascontiguousarray(w1[:, perm]).astype(np.float16),
                   "wal": wal1.astype(np.float16)} for k in range(C)]
        ncP0 = self.kernel("P0", c_in=IN_C, m_h=HC, m_al=2 * HEADS,
                           elu=False, bias_in=False)
        resP0 = self._run("P0", ncP0, mapsP0)
        h1 = np.ascontiguousarray(
            np.concatenate([r["hT"] for r in resP0], axis=1).T)  # [Np,HC] f16
        al1 = np.concatenate([r["alT"] for r in resP0], axis=1)  # [16,Np] f16
        als1 = np.ascontiguousarray(al1[:HEADS].T)
        ald1 = np.ascontiguousarray(al1[HEADS:].T)

        # ---- E1: layer-1 edge aggregation ----------------------------
        mapsE1 = [{"hsrc": g.stream_feat(h1, k),
                   "s8": g.stream_sel(k),
                   "zs": g.stream_zs(als1, ald1, k)} for k in range(C)]
        ncE1 = self.kernel("E1")
        resE1 = self._run("E1", ncE1, mapsE1)
        out1 = np.concatenate([r["out"] for r in resE1], axis=0)  # [Np, HC]

        # ---- P2: ELU + per-node h2 / logits --------------------------
        o1T = np.ascontiguousarray(out1.T)  # [HC, Np] f16, (c,h) rows
        w2 = np.asarray(W2, np.float32)
        wal2 = np.concatenate(
            [_fold_att(w2, np.asarray(a_src2, np.float32)),
             _fold_att(w2, np.asarray(a_dst2, np.float32))], axis=1)
        b1nz = bool(np.any(np.asarray(b1)))
        w2all = np.concatenate([w2[perm], wal2[perm]], axis=1)  # [HC, 66]
        mapsP2 = []
        for k in range(C):
            m = {"xT": np.ascontiguousarray(o1T[:, k * SH:(k + 1) * SH]),
                 "w": w2all.astype(np.float16)}
            if b1nz:
                m["bvec"] = np.asarray(b1, np.float32)[perm].reshape(HC, 1)
            mapsP2.append(m)
        ncP2 = self.kernel("P2", c_in=HC, m_h=OUT_C, m_al=2, elu=True,
                           bias_in=b1nz)
        resP2 = self._run("P2", ncP2, mapsP2)
        h2al = np.concatenate([r["hT"] for r in resP2], axis=1)  # [66, Np]
        h2 = np.ascontiguousarray(h2al[:OUT_C].T)  # [Np, 64] f16
        als2, ald2 = h2al[OUT_C], h2al[OUT_C + 1]

        # ---- E2: layer-2 edge aggregation ----------------------------
        b2nz = bool(np.any(np.asarray(b2)))
        mapsE2 = []
        for k in range(C):
            m = {"hsrc": g.stream_feat(h2, k, ones_col=True),
                 "s8": g.stream_sel(k),
                 "zs": g.stream_zs2(als2, ald2, k)}
            if b2nz:
                m["brep"] = np.tile(np.asarray(b2, np.float32), (P, 1))
            mapsE2.append(m)
        ncE2 = self.kernel("E2", bias_out=b2nz)
        resE2 = self._run("E2", ncE2, mapsE2)
        out2 = np.concatenate([r["out"] for r in resE2], axis=0)
        return out2[:N]


_RUNNER = _GatRunner()


def kernel(x, edge_index, W1, a_src1, a_dst1, b1, W2, a_src2, a_dst2, b2):
    """Full-input / full-output entry point. Returns [N, OUT_C] float32."""
    args = [np.asarray(v) for v in
            (x, edge_index, W1, a_src1, a_dst1, b1, W2, a_src2, a_dst2, b2)]
    return _RUNNER.run(*args).astype(np.float32)
